# revision 1
# baseline (speedup 1.0000x reference)
"""Trainium2 Bass kernel for AttentiveMinkUNetDiff KNN+MLP block (v2).

Self-contained: hardcodes shapes N=16384, M=32768, K=8, C=256, 8 cores.
Sharding: nodes across 8 cores; cond set replicated.

Per core (2048 nodes, 16 tiles of 128):
  1. PE: exact bf16-split integer matmul (K=19 rows) producing a
     per-node-ranking-equivalent of -40000*d^2 for all 32768 cond points.
  2. ACT copies PSUM->SBUF row buffers; DVE max/max_index per 8192-wide
     super -> 32-candidate pool with within-super indices.
  3. Top-16 of pool by value (match_replace rounds), then re-sorted by
     ascending cond index (jax.lax.top_k tie order) via masked-max trick.
  4. One packed indirect-DMA gather per candidate ([coords|feats] rows);
     exact d^2 recomputed bit-exactly vs XLA's fused fma chain (Dekker).
  5. Final 8 by exact value; inverse-distance weights; weighted mean of
     feats via ACT scaling + PE transpose-accumulate (weights sum to 1 so
     the mean commutes with W_proj); 3-layer MLP in transposed space;
     timestep-embedding branch folded into the final bias.
"""
import math
import numpy as np
import ml_dtypes

import concourse.bass as bass
import concourse.mybir as mybir
from concourse.tile import TileContext
from concourse import bass_utils
from concourse import bacc

bf16 = ml_dtypes.bfloat16
f32 = np.float32
AF = mybir.ActivationFunctionType
OP = mybir.AluOpType

N, M, K = 16384, 32768, 8
C = 256
PACK = C + 4                  # packed row: [part_c(4) | feats(256)]
EMBED, HALF = 96, 48
NCORES = 8
NSHARD = N // NCORES          # 2048
NTILES = NSHARD // 128        # 16
SUP = 4096                    # super-chunk width scanned from SBUF
NSUP = M // SUP               # 8
POOL = NSUP * 8               # 64
NCAND = 16
PI = float(np.pi)


# ---------------------------------------------------------------- host prep
def _split_rows(nodes, conds):
    """Build the 19 bf16-exact contraction rows. Validated vs reference."""
    a = nodes[:, 1:4].astype(np.int64)
    b = conds[:, 1:4].astype(np.int64)
    ah, al = a >> 5, a & 31
    bh, bl = b >> 5, b & 31
    lhs, rhs = [], []
    for k in range(3):
        lhs += [1280.0 * ah[:, k], 1280.0 * ah[:, k], 40.0 * al[:, k], 40.0 * al[:, k]]
        rhs += [32.0 * bh[:, k], 1.0 * bl[:, k], 32.0 * bh[:, k], 1.0 * bl[:, k]]
    B_total = (4 * b * b - 316 * b).sum(1) + 32768
    s2, s1, s0 = B_total >> 16, (B_total >> 8) & 255, B_total & 255
    nones = -np.ones(a.shape[0])
    lhs += [nones, nones, nones]
    rhs += [s2 * 65536.0, s1 * 256.0, s0 * 1.0]
    C_i = ((10 * a + 79) ** 2).sum(1) - 32768
    c3 = np.floor(C_i / 2 ** 21).astype(np.int64)
    r = C_i - c3 * 2 ** 21
    c2, c1, c0 = r >> 13, (r >> 5) & 255, r & 31
    mones = np.ones(b.shape[0])
    lhs += [-c3 * 2097152.0, -c2 * 8192.0, -c1 * 32.0, -c0 * 1.0]
    rhs += [mones, mones, mones, mones]
    LHS = np.stack(lhs).astype(f32)   # [19, N]
    RHS = np.stack(rhs).astype(f32)   # [19, M]
    return LHS.astype(bf16), RHS.astype(bf16)


def _transform(coords, stride, voxel, mc):
    c = coords.astype(np.float32)
    batch = (c[:, :1] * f32(mc * f32(2.0))).astype(f32)
    xyz = ((c[:, 1:] + f32(stride / 2.0)).astype(f32) * f32(voxel)).astype(f32)
    return np.concatenate([batch, xyz], 1).astype(f32)


def _pack_w(w):
    """W [dout, din] -> lhsT pack [128, 4*128]: col block (ct*2+dt)."""
    wt = np.ascontiguousarray(w.T.astype(f32))          # [din, dout]
    p = wt.reshape(2, 128, 2, 128)                      # [ct, c, dt, d]
    p = p.transpose(1, 0, 2, 3).reshape(128, 512)
    return np.ascontiguousarray(p)


_CACHE = {}


def _build_program():
    if 'nc' in _CACHE:
        return _CACHE['nc']
    nc = bacc.Bacc("TRN2", target_bir_lowering=False, debug=False,
                   num_devices=NCORES)
    dt = mybir.dt

    def din(name, shape, dtype):
        return nc.dram_tensor(name, shape, dtype, kind="ExternalInput").ap()

    lhsT = din('lhsT', [19, NSHARD], dt.bfloat16)
    rhs = din('rhsT', [19, M], dt.bfloat16)
    nodex = din('nodex', [128, NTILES * 3], dt.float32)
    packed = din('packed', [M, PACK], dt.float32)
    invbase = din('invbase', [128, POOL], dt.float32)
    eye = din('eye', [128, 128], dt.float32)
    wp = din('wp', [128, 512], dt.float32)
    wl1 = din('wl1', [128, 512], dt.float32)
    wl2 = din('wl2', [128, 512], dt.float32)
    bproj = din('bproj', [128, 2], dt.float32)
    bl1 = din('bl1', [128, 2], dt.float32)
    bcomb = din('bcomb', [128, 2], dt.float32)
    wt1 = din('wt1', [EMBED, EMBED], dt.float32)
    wt2 = din('wt2', [EMBED, C], dt.float32)
    bt1 = din('bt1', [EMBED, 1], dt.float32)
    freqs = din('freqs', [EMBED, 1], dt.float32)
    shifts = din('shifts', [EMBED, 1], dt.float32)
    tval = din('tval', [EMBED, 1], dt.float32)
    out = nc.dram_tensor('out', [NSHARD, C], dt.float32, kind="ExternalOutput").ap()

    with TileContext(nc) as tc, \
            tc.tile_pool(name="const", bufs=1) as cpool, \
            tc.tile_pool(name="work", bufs=2) as wpool, \
            tc.tile_pool(name="psum", bufs=2, space="PSUM") as ppool:

        # ---- constants to SBUF
        rhs_sb = cpool.tile([19, M], dt.bfloat16, tag="rhs")
        for j in range(8):
            nc.sync.dma_start(out=rhs_sb[:, j * (M // 8):(j + 1) * (M // 8)],
                              in_=rhs[:, j * (M // 8):(j + 1) * (M // 8)])
        lhs_sb = cpool.tile([19, NSHARD], dt.bfloat16, tag="lhs")
        nc.sync.dma_start(out=lhs_sb[:], in_=lhsT)
        nodex_sb = cpool.tile([128, NTILES * 3], dt.float32, tag="nodex")
        nc.sync.dma_start(out=nodex_sb[:], in_=nodex)
        invb_sb = cpool.tile([128, POOL], dt.float32, tag="invb")
        nc.sync.dma_start(out=invb_sb[:], in_=invbase)
        eye_sb = cpool.tile([128, 128], dt.float32, tag="eye")
        nc.sync.dma_start(out=eye_sb[:], in_=eye)
        wp_sb = cpool.tile([128, 512], dt.float32, tag="wp")
        nc.sync.dma_start(out=wp_sb[:], in_=wp)
        wl1_sb = cpool.tile([128, 512], dt.float32, tag="wl1")
        nc.sync.dma_start(out=wl1_sb[:], in_=wl1)
        wl2_sb = cpool.tile([128, 512], dt.float32, tag="wl2")
        nc.sync.dma_start(out=wl2_sb[:], in_=wl2)
        bproj_sb = cpool.tile([128, 2], dt.float32, tag="bproj")
        nc.sync.dma_start(out=bproj_sb[:], in_=bproj)
        bl1_sb = cpool.tile([128, 2], dt.float32, tag="bl1")
        nc.sync.dma_start(out=bl1_sb[:], in_=bl1)
        bcomb_sb = cpool.tile([128, 2], dt.float32, tag="bcomb")
        nc.sync.dma_start(out=bcomb_sb[:], in_=bcomb)
        wt1_sb = cpool.tile([EMBED, EMBED], dt.float32, tag="wt1")
        nc.sync.dma_start(out=wt1_sb[:], in_=wt1)
        wt2_sb = cpool.tile([EMBED, C], dt.float32, tag="wt2")
        nc.sync.dma_start(out=wt2_sb[:], in_=wt2)
        bt1_sb = cpool.tile([EMBED, 1], dt.float32, tag="bt1")
        nc.sync.dma_start(out=bt1_sb[:], in_=bt1)
        fr_sb = cpool.tile([EMBED, 1], dt.float32, tag="fr")
        nc.sync.dma_start(out=fr_sb[:], in_=freqs)
        sh_sb = cpool.tile([EMBED, 1], dt.float32, tag="sh")
        nc.sync.dma_start(out=sh_sb[:], in_=shifts)
        t_sb = cpool.tile([EMBED, 1], dt.float32, tag="t1x1")
        nc.sync.dma_start(out=t_sb[:], in_=tval)

        # ---- t branch -> fincol [128, 2]
        e = cpool.tile([EMBED, 1], dt.float32, tag="e")
        nc.vector.tensor_mul(e[:], t_sb[:], fr_sb[:])
        nc.vector.tensor_add(e[:], e[:], sh_sb[:])
        ki = cpool.tile([EMBED, 1], dt.int32, tag="ki")
        kf = cpool.tile([EMBED, 1], dt.float32, tag="kf")
        nc.vector.tensor_scalar(kf[:], e[:], float(1.0 / (2 * PI)), None, op0=OP.mult)
        nc.vector.tensor_copy(out=ki[:], in_=kf[:])
        nc.vector.tensor_copy(out=kf[:], in_=ki[:])
        nc.vector.tensor_scalar(kf[:], kf[:], float(2 * PI), None, op0=OP.mult)
        nc.vector.tensor_sub(e[:], e[:], kf[:])
        gt = cpool.tile([EMBED, 1], dt.float32, tag="gt")
        nc.vector.tensor_scalar(gt[:], e[:], float(PI), None, op0=OP.is_gt)
        nc.vector.tensor_scalar(gt[:], gt[:], float(2 * PI), None, op0=OP.mult)
        nc.vector.tensor_sub(e[:], e[:], gt[:])
        emb_sb = cpool.tile([EMBED, 1], dt.float32, tag="emb")
        nc.scalar.activation(emb_sb[:], e[:], AF.Sin)
        ps_t1 = ppool.tile([EMBED, 1], dt.float32, tag="mm")
        nc.tensor.matmul(ps_t1[:], lhsT=wt1_sb[:], rhs=emb_sb[:], start=True, stop=True)
        h96 = cpool.tile([EMBED, 1], dt.float32, tag="h96")
        nc.scalar.activation(h96[:], ps_t1[:], AF.Identity, bias=bt1_sb[:, 0:1])
        h96b = cpool.tile([EMBED, 1], dt.float32, tag="h96b")
        nc.vector.tensor_scalar(h96b[:], h96[:], 0.1, None, op0=OP.mult)
        nc.vector.tensor_max(h96b[:], h96b[:], h96[:])
        fincol = cpool.tile([128, 2], dt.float32, tag="fincol")
        for d in range(2):
            ps_t2 = ppool.tile([128, 1], dt.float32, tag="mm")
            nc.tensor.matmul(ps_t2[:], lhsT=wt2_sb[:, d * 128:(d + 1) * 128],
                             rhs=h96b[:], start=True, stop=True)
            nc.scalar.activation(fincol[:, d:d + 1], ps_t2[:], AF.Identity,
                                 bias=bcomb_sb[:, d:d + 1])

        c32k = cpool.tile([128, NCAND], dt.float32, tag="c32k")
        nc.vector.memset(c32k[:], 32768.0)

        # ---- main loop, software-pipelined: A(t+1) emitted before B(t)
        def stageA(t):
            st = {}
            lt = lhs_sb[:, t * 128:(t + 1) * 128]
            pool_vals = wpool.tile([128, POOL], dt.float32, tag="pvals", name="pvals", bufs=3)
            pool_lidx = wpool.tile([128, POOL], dt.uint16, tag="plidx", name="plidx", bufs=3)
            for s in range(NSUP):
                rowb = wpool.tile([128, SUP], dt.float32, tag="rowb", name="rowb", bufs=3)
                for h in range(SUP // 1024):
                    ps_d = ppool.tile([128, 1024], dt.float32, tag="dist", name="psd")
                    for q in range(2):
                        c0 = s * SUP + h * 1024 + q * 512
                        nc.tensor.matmul(ps_d[:, q * 512:(q + 1) * 512], lhsT=lt,
                                         rhs=rhs_sb[:, c0:c0 + 512],
                                         start=True, stop=True)
                    nc.scalar.activation(rowb[:, h * 1024:(h + 1) * 1024], ps_d[:],
                                         AF.Identity)
                nc.vector.max(out=pool_vals[:, s * 8:(s + 1) * 8], in_=rowb[:])
                nc.vector.max_index(out=pool_lidx[:, s * 8:(s + 1) * 8],
                                    in_max=pool_vals[:, s * 8:(s + 1) * 8],
                                    in_values=rowb[:])
            lidxf = wpool.tile([128, POOL], dt.float32, tag="lidxf", name="lidxf")
            nc.vector.tensor_copy(out=lidxf[:], in_=pool_lidx[:])
            pinv = wpool.tile([128, POOL], dt.float32, tag="pinv", name="pinv")
            nc.vector.tensor_sub(pinv[:], invb_sb[:], lidxf[:])
            pv2 = wpool.tile([128, POOL], dt.float32, tag="pv2", name="pv2")
            pv3 = wpool.tile([128, POOL], dt.float32, tag="pv3", name="pv3")
            v8 = wpool.tile([128, 8], dt.float32, tag="v8", name="v8")
            nc.vector.max(out=v8[:], in_=pool_vals[:])
            nc.vector.match_replace(out=pv2[:], in_to_replace=v8[:],
                                    in_values=pool_vals[:], imm_value=-3e38)
            nc.vector.max(out=v8[:], in_=pv2[:])
            nc.vector.match_replace(out=pv3[:], in_to_replace=v8[:],
                                    in_values=pv2[:], imm_value=-3e38)
            maskp = wpool.tile([128, POOL], dt.float32, tag="maskp", name="maskp")
            nc.vector.tensor_tensor(out=maskp[:], in0=pv3[:], in1=pool_vals[:],
                                    op=OP.not_equal)
            nc.vector.tensor_mul(maskp[:], maskp[:], pinv[:])
            inv16 = wpool.tile([128, NCAND], dt.float32, tag="inv16", name="inv16")
            mv2 = wpool.tile([128, POOL], dt.float32, tag="mv2", name="mv2")
            nc.vector.max(out=inv16[:, 0:8], in_=maskp[:])
            nc.vector.match_replace(out=mv2[:], in_to_replace=inv16[:, 0:8],
                                    in_values=maskp[:], imm_value=0.0)
            nc.vector.max(out=inv16[:, 8:16], in_=mv2[:])
            g16f = wpool.tile([128, NCAND], dt.float32, tag="g16f", name="g16f")
            nc.vector.tensor_sub(g16f[:], c32k[:], inv16[:])
            g16u = wpool.tile([128, NCAND], dt.uint32, tag="g16u", name="g16u")
            nc.vector.tensor_copy(out=g16u[:], in_=g16f[:])
            gp = wpool.tile([128, NCAND, PACK], dt.float32, tag="gp", name="gp",
                            bufs=3)
            for k in range(NCAND):
                nc.gpsimd.indirect_dma_start(
                    out=gp[:, k, :], out_offset=None, in_=packed,
                    in_offset=bass.IndirectOffsetOnAxis(ap=g16u[:, k:k + 1], axis=0))
            st['gp'] = gp
            return st

        def stageB(t, st):
            gp = st['gp']
            xs = [nodex_sb[:, t * 3 + k: t * 3 + k + 1] for k in range(3)]
            dcol = [wpool.tile([128, NCAND], dt.float32, tag=f"d{k}",
                               name=f"dcol{k}") for k in range(3)]
            for k in range(3):
                nc.vector.tensor_scalar(dcol[k][:], gp[:, :, k + 1], xs[k],
                                        None, op0=OP.subtract)
            acc = wpool.tile([128, NCAND], dt.float32, tag="acc", name="acc")
            nc.gpsimd.tensor_mul(acc[:], dcol[0][:], dcol[0][:])
            tt = [wpool.tile([128, NCAND], dt.float32, tag=f"t{i}",
                             name=f"tt{i}") for i in range(6)]
            for k in (1, 2):
                d = dcol[k]
                T0, T1, T2, T3, T4, T5 = tt
                nc.vector.tensor_scalar(T0[:], d[:], 4097.0, None, op0=OP.mult)
                nc.gpsimd.tensor_sub(T1[:], T0[:], d[:])
                nc.gpsimd.tensor_sub(T0[:], T0[:], T1[:])
                nc.gpsimd.tensor_sub(T1[:], d[:], T0[:])
                nc.gpsimd.tensor_mul(T2[:], d[:], d[:])
                nc.gpsimd.tensor_mul(T3[:], T0[:], T0[:])
                nc.gpsimd.tensor_sub(T3[:], T3[:], T2[:])
                nc.gpsimd.tensor_add(T4[:], T1[:], T1[:])
                nc.gpsimd.tensor_mul(T4[:], T0[:], T4[:])
                nc.gpsimd.tensor_add(T3[:], T3[:], T4[:])
                nc.gpsimd.tensor_mul(T4[:], T1[:], T1[:])
                nc.gpsimd.tensor_add(T3[:], T3[:], T4[:])
                nc.gpsimd.tensor_add(T4[:], T2[:], acc[:])
                nc.gpsimd.tensor_sub(T5[:], T4[:], T2[:])
                nc.gpsimd.tensor_sub(T0[:], T4[:], T5[:])
                nc.gpsimd.tensor_sub(T0[:], T2[:], T0[:])
                nc.gpsimd.tensor_sub(T1[:], acc[:], T5[:])
                nc.gpsimd.tensor_add(T0[:], T0[:], T1[:])
                nc.gpsimd.tensor_add(T0[:], T0[:], T3[:])
                nc.gpsimd.tensor_add(acc[:], T4[:], T0[:])
            nd2 = wpool.tile([128, NCAND], dt.float32, tag="nd2", name="nd2")
            nc.vector.tensor_scalar(nd2[:], acc[:], -1.0, None, op0=OP.mult)
            v8f = wpool.tile([128, 8], dt.float32, tag="v8f", name="v8f")
            nc.vector.max(out=v8f[:], in_=nd2[:])
            refb = wpool.tile([128, NCAND], dt.float32, tag="refb", name="refb")
            nc.vector.match_replace(out=refb[:], in_to_replace=v8f[:],
                                    in_values=nd2[:], imm_value=3e38)
            mask2 = wpool.tile([128, NCAND], dt.float32, tag="mask2", name="mask2")
            nc.vector.tensor_tensor(out=mask2[:], in0=refb[:], in1=nd2[:],
                                    op=OP.not_equal)
            dist = wpool.tile([128, NCAND], dt.float32, tag="dist16", name="dist16")
            nc.scalar.activation(dist[:], acc[:], AF.Sqrt)
            nc.vector.tensor_scalar_max(dist[:], dist[:], 1e-6)
            wr = wpool.tile([128, NCAND], dt.float32, tag="wr", name="wr")
            nc.vector.reciprocal(out=wr[:], in_=dist[:])
            nc.vector.tensor_mul(wr[:], wr[:], mask2[:])
            wsum = wpool.tile([128, 1], dt.float32, tag="wsum", name="wsum")
            nc.vector.tensor_reduce(out=wsum[:], in_=wr[:],
                                    axis=mybir.AxisListType.X, op=OP.add)
            wsr = wpool.tile([128, 1], dt.float32, tag="wsr", name="wsr")
            nc.vector.reciprocal(out=wsr[:], in_=wsum[:])
            wn = wpool.tile([128, NCAND], dt.float32, tag="wn", name="wn")
            nc.vector.tensor_scalar(wn[:], wr[:], wsr[:, 0:1], None, op0=OP.mult)

            ps_fT = ppool.tile([128, C], dt.float32, tag="tr", name="psft")
            for k in range(NCAND):
                sc = wpool.tile([128, C], dt.float32, tag="sc", name="sc", bufs=4)
                nc.scalar.activation(sc[:], gp[:, k, 4:4 + C], AF.Identity,
                                     scale=wn[:, k:k + 1])
                for half_i in range(2):
                    nc.tensor.matmul(
                        ps_fT[:, half_i * 128:(half_i + 1) * 128],
                        lhsT=sc[:, half_i * 128:(half_i + 1) * 128],
                        rhs=eye_sb[:], is_transpose=True,
                        start=(k == 0 and half_i == 0),
                        stop=(k == NCAND - 1 and half_i == 1))
            fT = wpool.tile([128, C], dt.float32, tag="fT", name="fT")
            nc.scalar.activation(fT[:], ps_fT[:], AF.Identity)

            def dense(src_sb, wpack, bias_sb, leaky, outtag):
                o = wpool.tile([128, C], dt.float32, tag=outtag, name=outtag)
                for d in range(2):
                    ps = ppool.tile([128, 128], dt.float32, tag="mm", name="psmm")
                    for ct in range(2):
                        nc.tensor.matmul(
                            ps[:], lhsT=wpack[:, (ct * 2 + d) * 128:(ct * 2 + d + 1) * 128],
                            rhs=src_sb[:, ct * 128:(ct + 1) * 128],
                            start=(ct == 0), stop=(ct == 1))
                    nc.scalar.activation(o[:, d * 128:(d + 1) * 128], ps[:],
                                         AF.Identity, bias=bias_sb[:, d:d + 1])
                if leaky:
                    tmp = wpool.tile([128, C], dt.float32, tag=outtag + "lk",
                                     name=outtag + "lk")
                    nc.vector.tensor_scalar(tmp[:], o[:], 0.1, None, op0=OP.mult)
                    nc.vector.tensor_max(o[:], o[:], tmp[:])
                return o

            mT = dense(fT, wp_sb, bproj_sb, False, "mT")
            h1T = dense(mT, wl1_sb, bl1_sb, True, "h1T")
            pT = dense(h1T, wl2_sb, fincol, False, "pT")
            osb = wpool.tile([128, C], dt.float32, tag="osb", name="osb")
            for dth in range(2):
                ps_tr = ppool.tile([128, 128], dt.float32, tag="mm", name="pstr")
                nc.tensor.matmul(ps_tr[:], lhsT=pT[:, dth * 128:(dth + 1) * 128],
                                 rhs=eye_sb[:], is_transpose=True,
                                 start=True, stop=True)
                nc.scalar.activation(osb[:, dth * 128:(dth + 1) * 128], ps_tr[:],
                                     AF.Identity)
            nc.sync.dma_start(out=out[t * 128:(t + 1) * 128, :], in_=osb[:])

        pending = None
        for t in range(NTILES + 1):
            if t < NTILES:
                st = stageA(t)
            if pending is not None:
                stageB(t - 1, pending)
            pending = st if t < NTILES else None

    nc.compile()
    _CACHE['nc'] = nc
    return nc


# ---------------------------------------------------------------- host entry
def kernel(node_coords, cond_coords, cond_feats, t,
           W_proj, b_proj, W_l1, b_l1, W_l2, b_l2, W_t1, b_t1, W_t2, b_t2):
    node_coords = np.asarray(node_coords)
    cond_coords = np.asarray(cond_coords)
    cond_feats = np.asarray(cond_feats, dtype=np.float32)
    mc = np.float32(node_coords.astype(np.float32).max())
    part_c = _transform(cond_coords, 1.0, 0.01, mc)
    packed = np.ascontiguousarray(np.concatenate([part_c, cond_feats], 1))
    LHS, RHS = _split_rows(node_coords, cond_coords)
    full_c = _transform(node_coords, 16.0, 0.05, mc)

    invbase = np.tile((32768.0 - (np.arange(POOL) // 8) * SUP).astype(f32)[None, :],
                      (128, 1))
    eye = np.eye(128, dtype=f32)
    freqs1 = np.exp(np.arange(HALF, dtype=np.float32) *
                    f32(-math.log(10000.0) / (HALF - 1))).astype(f32)
    freqs = np.concatenate([freqs1, freqs1])
    shifts = np.concatenate([np.zeros(HALF, f32), np.full(HALF, PI / 2, f32)])

    nc = _build_program()
    in_maps = []
    for i in range(NCORES):
        sl = slice(i * NSHARD, (i + 1) * NSHARD)
        nodex = np.ascontiguousarray(
            full_c[sl, 1:4].reshape(NTILES, 128, 3).transpose(1, 0, 2)
            .reshape(128, NTILES * 3))
        in_maps.append({
            'lhsT': np.ascontiguousarray(LHS[:, sl]),
            'rhsT': RHS,
            'nodex': nodex,
            'packed': packed,
            'invbase': invbase,
            'eye': eye,
            'wp': _pack_w(np.asarray(W_proj, dtype=f32)),
            'wl1': _pack_w(np.asarray(W_l1, dtype=f32)),
            'wl2': _pack_w(np.asarray(W_l2, dtype=f32)),
            'bproj': np.asarray(b_proj, f32).reshape(2, 128).T.copy(),
            'bl1': np.asarray(b_l1, f32).reshape(2, 128).T.copy(),
            'bcomb': (np.asarray(b_l2, f32) + np.asarray(b_t2, f32)).reshape(2, 128).T.copy(),
            'wt1': np.ascontiguousarray(np.asarray(W_t1, f32).T),
            'wt2': np.ascontiguousarray(np.asarray(W_t2, f32).T),
            'bt1': np.asarray(b_t1, f32).reshape(EMBED, 1).copy(),
            'freqs': freqs.reshape(EMBED, 1).copy(),
            'shifts': shifts.reshape(EMBED, 1).copy(),
            'tval': np.full((EMBED, 1), np.asarray(t, f32).reshape(()), f32),
        })
    res = bass_utils.run_bass_kernel_spmd(nc, in_maps, core_ids=list(range(NCORES)))
    _CACHE['last_result'] = res
    outs = [res.results[i]['out'] for i in range(NCORES)]
    return np.concatenate(outs, 0)



# revision 2
# speedup vs baseline: 1.9064x; 1.9064x over previous
"""Trainium2 Bass kernel for AttentiveMinkUNetDiff KNN+MLP block (v3).

Self-contained: hardcodes shapes N=16384, M=32768, K=8, C=256, 8 cores.

v3 strategy (windowed scan):
  Host: Morton-sort cond points; blocks of 512 consecutive sorted points.
  Nodes sorted by Morton of their box-clamped position; tiles of 128
  nodes. Per tile, select the cond blocks whose bbox is within a
  rigorous per-node footprint radius rr = sqrt(2*D*delta + delta^2)
  (delta from a density bound) of any member's clamped position -- this
  provably covers every node's kNN ball. Selected blocks are host-packed
  into a contiguous per-tile window (budget per slot, filler columns
  score strictly below every real point). Tiles are assigned to
  (core, slot) by descending window size so all cores share one
  per-slot budget schedule.

  Device per tile: 19-row exact bf16-split integer matmul over the
  window only; DVE MAX8/FIND_INDEX8 per 512-granule -> top-8-per-granule
  pool; top-12 candidates by value, reordered by ascending cond index;
  indirect-DMA gather of packed [coords|feats] rows; exact d^2 recompute
  (Dekker, matches XLA fma chain); final 8 by exact value; inverse-
  distance weights; weighted feature mean via DVE scalar_tensor_tensor
  chain; transpose; 3-layer MLP in transposed space; timestep branch
  folded into the final bias.
"""
import math
import numpy as np
import ml_dtypes

import concourse.bass as bass
import concourse.mybir as mybir
from concourse.tile import TileContext
from concourse import bass_utils
from concourse import bacc

bf16 = ml_dtypes.bfloat16
f32 = np.float32
AF = mybir.ActivationFunctionType
OP = mybir.AluOpType

N, M, K = 16384, 32768, 8
C = 256
PACK = C + 4                  # packed row: [part_c(4) | feats(256)]
EMBED, HALF = 96, 48
NCORES = 8
NSHARD = N // NCORES          # 2048
NTILES = NSHARD // 128        # 16
BS = 512                      # cond block / scan granule width
NB = M // BS                  # 64 blocks
CSAFE = 40.0
NCAND = 12
PI = float(np.pi)
NEG = -3.0e38


# ---------------------------------------------------------------- host prep
def _split_rows(nodes, conds):
    """Build the 19 bf16-exact contraction rows. Validated vs reference."""
    a = nodes[:, 1:4].astype(np.int64)
    b = conds[:, 1:4].astype(np.int64)
    ah, al = a >> 5, a & 31
    bh, bl = b >> 5, b & 31
    lhs, rhs = [], []
    for k in range(3):
        lhs += [1280.0 * ah[:, k], 1280.0 * ah[:, k], 40.0 * al[:, k], 40.0 * al[:, k]]
        rhs += [32.0 * bh[:, k], 1.0 * bl[:, k], 32.0 * bh[:, k], 1.0 * bl[:, k]]
    B_total = (4 * b * b - 316 * b).sum(1) + 32768
    s2, s1, s0 = B_total >> 16, (B_total >> 8) & 255, B_total & 255
    nones = -np.ones(a.shape[0])
    lhs += [nones, nones, nones]
    rhs += [s2 * 65536.0, s1 * 256.0, s0 * 1.0]
    C_i = ((10 * a + 79) ** 2).sum(1) - 32768
    c3 = np.floor(C_i / 2 ** 21).astype(np.int64)
    r = C_i - c3 * 2 ** 21
    c2, c1, c0 = r >> 13, (r >> 5) & 255, r & 31
    mones = np.ones(b.shape[0])
    lhs += [-c3 * 2097152.0, -c2 * 8192.0, -c1 * 32.0, -c0 * 1.0]
    rhs += [mones, mones, mones, mones]
    LHS = np.stack(lhs).astype(f32)   # [19, N]
    RHS = np.stack(rhs).astype(f32)   # [19, M]
    return LHS.astype(bf16), RHS.astype(bf16)


def _filler_col():
    v = np.zeros(19, f32)
    v[12], v[13], v[14] = 255.0 * 65536.0, 255.0 * 256.0, 255.0
    v[15:19] = 1.0
    return v.astype(bf16)


def _transform(coords, stride, voxel, mc):
    c = coords.astype(np.float32)
    batch = (c[:, :1] * f32(mc * f32(2.0))).astype(f32)
    xyz = ((c[:, 1:] + f32(stride / 2.0)).astype(f32) * f32(voxel)).astype(f32)
    return np.concatenate([batch, xyz], 1).astype(f32)


def _pack_w(w):
    """W [dout, din] -> lhsT pack [128, 4*128]: col block (ct*2+dt)."""
    wt = np.ascontiguousarray(w.T.astype(f32))          # [din, dout]
    p = wt.reshape(2, 128, 2, 128)                      # [ct, c, dt, d]
    p = p.transpose(1, 0, 2, 3).reshape(128, 512)
    return np.ascontiguousarray(p)


def _morton3(u):
    u = u.astype(np.uint64)

    def spread(v):
        v = v & 0x3ff
        v = (v | (v << 16)) & 0x030000FF
        v = (v | (v << 8)) & 0x0300F00F
        v = (v | (v << 4)) & 0x030C30C3
        v = (v | (v << 2)) & 0x09249249
        return v
    return (spread(u[:, 0]) << 2) | (spread(u[:, 1]) << 1) | spread(u[:, 2])


def _host_plan(node_coords, cond_coords):
    """Sort orders, per-tile block selection, slot schedule, assignment."""
    nx = (node_coords[:, 1:4].astype(np.float32) + f32(8.0)) * f32(0.05)
    cx = (cond_coords[:, 1:4].astype(np.float32) + f32(0.5)) * f32(0.01)
    ck = _morton3(cond_coords[:, 1:4])
    corder = np.argsort(ck, kind='stable')
    bmin, bmax = cx.min(0), cx.max(0)
    clamped = np.clip(nx, bmin, bmax)
    D = np.linalg.norm(nx - clamped, axis=1)
    dens = M / float(np.prod((bmax - bmin).astype(np.float64)))
    outdims = ((nx < bmin) | (nx > bmax)).sum(1)
    ceff = CSAFE * np.maximum(1.0, 2.0 ** (outdims - 1))
    vol = ceff / dens
    delta_face = np.sqrt(vol / (np.pi * np.maximum(D, 1e-9)))
    delta_sph = (vol * 3.0 / (4.0 * np.pi)) ** (1.0 / 3.0)
    delta = np.where(D > 0.3, delta_face, delta_sph)
    rr = np.sqrt(2 * D * delta + delta ** 2)

    ngrid = np.clip(np.round(clamped / 0.01 - 0.5), 0, 1023).astype(np.int64)
    nk = _morton3(ngrid)
    norder = np.argsort(nk, kind='stable')

    bidx = corder.reshape(NB, BS)
    bco = cx[bidx]
    blo, bhi = bco.min(1), bco.max(1)

    ntile = N // 128
    sel_blocks, w_tiles = [], []
    for t in range(ntile):
        mem = norder[t * 128:(t + 1) * 128]
        s, r = clamped[mem], rr[mem]
        d = (np.maximum(blo[None] - s[:, None], 0)
             + np.maximum(s[:, None] - bhi[None], 0))
        need = (np.linalg.norm(d, axis=2) <= r[:, None]).any(0)
        sel = np.where(need)[0]
        sel_blocks.append(sel)
        w_tiles.append(len(sel) * BS)
    w_tiles = np.array(w_tiles)
    pi = np.argsort(-w_tiles, kind='stable')      # tiles sorted by W desc
    budgets = tuple(int(w_tiles[pi[8 * k]]) for k in range(NTILES))
    # tile pi[8k+c] -> core c, slot k
    assign = pi.reshape(NTILES, NCORES)
    return dict(corder=corder, norder=norder, sel_blocks=sel_blocks,
                budgets=budgets, assign=assign)


_CACHE = {}


def _build_program(budgets):
    key = ('nc', budgets)
    if key in _CACHE:
        return _CACHE[key]
    nc = bacc.Bacc("TRN2", target_bir_lowering=False, debug=False,
                   num_devices=NCORES)
    dt = mybir.dt
    WMAX = max(budgets)
    WTOT = sum(budgets)
    pools = [b // 64 for b in budgets]             # pool width per slot
    PTOT = sum(pools)
    PMAX = max(pools)
    woff = np.cumsum([0] + [b for b in budgets])
    poff = np.cumsum([0] + pools)

    def din(name, shape, dtype):
        return nc.dram_tensor(name, shape, dtype, kind="ExternalInput").ap()

    lhsT = din('lhsT', [19, NSHARD], dt.bfloat16)
    rhsw = din('rhsw', [19, WTOT], dt.bfloat16)
    nodex = din('nodex', [128, NTILES * 3], dt.float32)
    packed = din('packed', [M, PACK], dt.float32)
    invbase = din('invbase', [128, PTOT], dt.float32)
    eye = din('eye', [128, 128], dt.float32)
    wp = din('wp', [128, 512], dt.float32)
    wl1 = din('wl1', [128, 512], dt.float32)
    wl2 = din('wl2', [128, 512], dt.float32)
    bproj = din('bproj', [128, 2], dt.float32)
    bl1 = din('bl1', [128, 2], dt.float32)
    bcomb = din('bcomb', [128, 2], dt.float32)
    wt1 = din('wt1', [EMBED, EMBED], dt.float32)
    wt2 = din('wt2', [EMBED, C], dt.float32)
    bt1 = din('bt1', [EMBED, 1], dt.float32)
    freqs = din('freqs', [EMBED, 1], dt.float32)
    shifts = din('shifts', [EMBED, 1], dt.float32)
    tval = din('tval', [EMBED, 1], dt.float32)
    out = nc.dram_tensor('out', [NSHARD, C], dt.float32, kind="ExternalOutput").ap()

    with TileContext(nc) as tc, \
            tc.tile_pool(name="const", bufs=1) as cpool, \
            tc.tile_pool(name="work", bufs=2) as wpool, \
            tc.tile_pool(name="psum", bufs=2, space="PSUM") as ppool:

        # ---- constants to SBUF
        lhs_sb = cpool.tile([19, NSHARD], dt.bfloat16, tag="lhs")
        nc.sync.dma_start(out=lhs_sb[:], in_=lhsT)
        nodex_sb = cpool.tile([128, NTILES * 3], dt.float32, tag="nodex")
        nc.sync.dma_start(out=nodex_sb[:], in_=nodex)
        invb_sb = cpool.tile([128, PTOT], dt.float32, tag="invb")
        nc.sync.dma_start(out=invb_sb[:], in_=invbase)
        eye_sb = cpool.tile([128, 128], dt.float32, tag="eye")
        nc.sync.dma_start(out=eye_sb[:], in_=eye)
        wp_sb = cpool.tile([128, 512], dt.float32, tag="wp")
        nc.sync.dma_start(out=wp_sb[:], in_=wp)
        wl1_sb = cpool.tile([128, 512], dt.float32, tag="wl1")
        nc.sync.dma_start(out=wl1_sb[:], in_=wl1)
        wl2_sb = cpool.tile([128, 512], dt.float32, tag="wl2")
        nc.sync.dma_start(out=wl2_sb[:], in_=wl2)
        bproj_sb = cpool.tile([128, 2], dt.float32, tag="bproj")
        nc.sync.dma_start(out=bproj_sb[:], in_=bproj)
        bl1_sb = cpool.tile([128, 2], dt.float32, tag="bl1")
        nc.sync.dma_start(out=bl1_sb[:], in_=bl1)
        bcomb_sb = cpool.tile([128, 2], dt.float32, tag="bcomb")
        nc.sync.dma_start(out=bcomb_sb[:], in_=bcomb)
        wt1_sb = cpool.tile([EMBED, EMBED], dt.float32, tag="wt1")
        nc.sync.dma_start(out=wt1_sb[:], in_=wt1)
        wt2_sb = cpool.tile([EMBED, C], dt.float32, tag="wt2")
        nc.sync.dma_start(out=wt2_sb[:], in_=wt2)
        bt1_sb = cpool.tile([EMBED, 1], dt.float32, tag="bt1")
        nc.sync.dma_start(out=bt1_sb[:], in_=bt1)
        fr_sb = cpool.tile([EMBED, 1], dt.float32, tag="fr")
        nc.sync.dma_start(out=fr_sb[:], in_=freqs)
        sh_sb = cpool.tile([EMBED, 1], dt.float32, tag="sh")
        nc.sync.dma_start(out=sh_sb[:], in_=shifts)
        t_sb = cpool.tile([EMBED, 1], dt.float32, tag="t1x1")
        nc.sync.dma_start(out=t_sb[:], in_=tval)

        # ---- t branch -> fincol [128, 2]
        e = cpool.tile([EMBED, 1], dt.float32, tag="e")
        nc.vector.tensor_mul(e[:], t_sb[:], fr_sb[:])
        nc.vector.tensor_add(e[:], e[:], sh_sb[:])
        ki = cpool.tile([EMBED, 1], dt.int32, tag="ki")
        kf = cpool.tile([EMBED, 1], dt.float32, tag="kf")
        nc.vector.tensor_scalar(kf[:], e[:], float(1.0 / (2 * PI)), None, op0=OP.mult)
        nc.vector.tensor_copy(out=ki[:], in_=kf[:])
        nc.vector.tensor_copy(out=kf[:], in_=ki[:])
        nc.vector.tensor_scalar(kf[:], kf[:], float(2 * PI), None, op0=OP.mult)
        nc.vector.tensor_sub(e[:], e[:], kf[:])
        gt = cpool.tile([EMBED, 1], dt.float32, tag="gt")
        nc.vector.tensor_scalar(gt[:], e[:], float(PI), None, op0=OP.is_gt)
        nc.vector.tensor_scalar(gt[:], gt[:], float(2 * PI), None, op0=OP.mult)
        nc.vector.tensor_sub(e[:], e[:], gt[:])
        emb_sb = cpool.tile([EMBED, 1], dt.float32, tag="emb")
        nc.scalar.activation(emb_sb[:], e[:], AF.Sin)
        ps_t1 = ppool.tile([EMBED, 1], dt.float32, tag="mm")
        nc.tensor.matmul(ps_t1[:], lhsT=wt1_sb[:], rhs=emb_sb[:], start=True, stop=True)
        h96 = cpool.tile([EMBED, 1], dt.float32, tag="h96")
        nc.scalar.activation(h96[:], ps_t1[:], AF.Identity, bias=bt1_sb[:, 0:1])
        h96b = cpool.tile([EMBED, 1], dt.float32, tag="h96b")
        nc.vector.tensor_scalar(h96b[:], h96[:], 0.1, None, op0=OP.mult)
        nc.vector.tensor_max(h96b[:], h96b[:], h96[:])
        fincol = cpool.tile([128, 2], dt.float32, tag="fincol")
        for d in range(2):
            ps_t2 = ppool.tile([128, 1], dt.float32, tag="mm")
            nc.tensor.matmul(ps_t2[:], lhsT=wt2_sb[:, d * 128:(d + 1) * 128],
                             rhs=h96b[:], start=True, stop=True)
            nc.scalar.activation(fincol[:, d:d + 1], ps_t2[:], AF.Identity,
                                 bias=bcomb_sb[:, d:d + 1])

        c32k = cpool.tile([128, NCAND], dt.float32, tag="c32k")
        nc.vector.memset(c32k[:], 32768.0)

        # ---- main loop, software-pipelined: A(t+1) emitted before B(t)
        def stageA(t):
            st = {}
            Wk = budgets[t]
            Gk = Wk // BS
            Pk = pools[t]
            lt = lhs_sb[:, t * 128:(t + 1) * 128]
            rhs_sb = wpool.tile([19, WMAX], dt.bfloat16, tag="rhsw", name="rhsw",
                                bufs=2)
            for j in range(0, Wk, 4096):
                w = min(4096, Wk - j)
                nc.sync.dma_start(out=rhs_sb[:, j:j + w],
                                  in_=rhsw[:, woff[t] + j:woff[t] + j + w])
            pool_vals = wpool.tile([128, PMAX], dt.float32, tag="pvals",
                                   name="pvals", bufs=2)
            pool_lidx = wpool.tile([128, PMAX], dt.uint16, tag="plidx",
                                   name="plidx", bufs=2)
            for g in range(Gk):
                ps_d = ppool.tile([128, BS], dt.float32, tag="dist", name="psd",
                                  bufs=4)
                nc.tensor.matmul(ps_d[:], lhsT=lt,
                                 rhs=rhs_sb[:, g * BS:(g + 1) * BS],
                                 start=True, stop=True)
                rowb = wpool.tile([128, BS], dt.float32, tag="rowb", name="rowb",
                                  bufs=6)
                nc.scalar.activation(rowb[:], ps_d[:], AF.Identity)
                nc.vector.max(out=pool_vals[:, g * 8:(g + 1) * 8], in_=rowb[:])
                nc.vector.max_index(out=pool_lidx[:, g * 8:(g + 1) * 8],
                                    in_max=pool_vals[:, g * 8:(g + 1) * 8],
                                    in_values=rowb[:])
            pv = pool_vals[:, 0:Pk]
            lidxf = wpool.tile([128, PMAX], dt.float32, tag="lidxf", name="lidxf")
            nc.vector.tensor_copy(out=lidxf[:, 0:Pk], in_=pool_lidx[:, 0:Pk])
            pinv = wpool.tile([128, PMAX], dt.float32, tag="pinv", name="pinv")
            nc.vector.tensor_sub(pinv[:, 0:Pk],
                                 invb_sb[:, poff[t]:poff[t] + Pk],
                                 lidxf[:, 0:Pk])
            # top-12 by value: v8 (ranks 1-8), v8b[:,0:4] (ranks 9-12)
            v8 = wpool.tile([128, 8], dt.float32, tag="v8", name="v8")
            pv2 = wpool.tile([128, PMAX], dt.float32, tag="pv2", name="pv2")
            pv3 = wpool.tile([128, PMAX], dt.float32, tag="pv3", name="pv3")
            nc.vector.max(out=v8[:], in_=pv)
            nc.vector.match_replace(out=pv2[:, 0:Pk], in_to_replace=v8[:],
                                    in_values=pv, imm_value=NEG)
            v8b = wpool.tile([128, 8], dt.float32, tag="v8b", name="v8b")
            nc.vector.max(out=v8b[:], in_=pv2[:, 0:Pk])
            v12p = wpool.tile([128, 8], dt.float32, tag="v12p", name="v12p")
            nc.vector.memset(v12p[:], NEG)
            nc.vector.tensor_copy(out=v12p[:, 0:4], in_=v8b[:, 0:4])
            nc.vector.match_replace(out=pv3[:, 0:Pk], in_to_replace=v12p[:],
                                    in_values=pv2[:, 0:Pk], imm_value=NEG)
            maskp = wpool.tile([128, PMAX], dt.float32, tag="maskp", name="maskp")
            nc.vector.tensor_tensor(out=maskp[:, 0:Pk], in0=pv3[:, 0:Pk],
                                    in1=pv, op=OP.not_equal)
            nc.vector.tensor_mul(maskp[:, 0:Pk], maskp[:, 0:Pk], pinv[:, 0:Pk])
            # order the 12 by ascending global index
            i8a = wpool.tile([128, 8], dt.float32, tag="i8a", name="i8a")
            nc.vector.max(out=i8a[:], in_=maskp[:, 0:Pk])
            mv2 = wpool.tile([128, PMAX], dt.float32, tag="mv2", name="mv2")
            nc.vector.match_replace(out=mv2[:, 0:Pk], in_to_replace=i8a[:],
                                    in_values=maskp[:, 0:Pk], imm_value=0.0)
            i8b = wpool.tile([128, 8], dt.float32, tag="i8b", name="i8b")
            nc.vector.max(out=i8b[:], in_=mv2[:, 0:Pk])
            inv12 = wpool.tile([128, NCAND], dt.float32, tag="inv12", name="inv12")
            nc.vector.tensor_copy(out=inv12[:, 0:8], in_=i8a[:])
            nc.vector.tensor_copy(out=inv12[:, 8:12], in_=i8b[:, 0:4])
            g12f = wpool.tile([128, NCAND], dt.float32, tag="g12f", name="g12f")
            nc.vector.tensor_sub(g12f[:], c32k[:], inv12[:])
            g12u = wpool.tile([128, NCAND], dt.uint32, tag="g12u", name="g12u")
            nc.vector.tensor_copy(out=g12u[:], in_=g12f[:])
            gp = wpool.tile([128, NCAND, PACK], dt.float32, tag="gp", name="gp",
                            bufs=3)
            for k in range(NCAND):
                nc.gpsimd.indirect_dma_start(
                    out=gp[:, k, :], out_offset=None, in_=packed,
                    in_offset=bass.IndirectOffsetOnAxis(ap=g12u[:, k:k + 1], axis=0))
            st['gp'] = gp
            return st

        def stageB(t, st):
            gp = st['gp']
            xs = [nodex_sb[:, t * 3 + k: t * 3 + k + 1] for k in range(3)]
            dcol = [wpool.tile([128, NCAND], dt.float32, tag=f"d{k}",
                               name=f"dcol{k}") for k in range(3)]
            for k in range(3):
                nc.vector.tensor_scalar(dcol[k][:], gp[:, :, k + 1], xs[k],
                                        None, op0=OP.subtract)
            acc = wpool.tile([128, NCAND], dt.float32, tag="acc", name="acc")
            nc.gpsimd.tensor_mul(acc[:], dcol[0][:], dcol[0][:])
            tt = [wpool.tile([128, NCAND], dt.float32, tag=f"t{i}",
                             name=f"tt{i}") for i in range(6)]
            for k in (1, 2):
                d = dcol[k]
                T0, T1, T2, T3, T4, T5 = tt
                nc.vector.tensor_scalar(T0[:], d[:], 4097.0, None, op0=OP.mult)
                nc.gpsimd.tensor_sub(T1[:], T0[:], d[:])
                nc.gpsimd.tensor_sub(T0[:], T0[:], T1[:])
                nc.gpsimd.tensor_sub(T1[:], d[:], T0[:])
                nc.gpsimd.tensor_mul(T2[:], d[:], d[:])
                nc.gpsimd.tensor_mul(T3[:], T0[:], T0[:])
                nc.gpsimd.tensor_sub(T3[:], T3[:], T2[:])
                nc.gpsimd.tensor_add(T4[:], T1[:], T1[:])
                nc.gpsimd.tensor_mul(T4[:], T0[:], T4[:])
                nc.gpsimd.tensor_add(T3[:], T3[:], T4[:])
                nc.gpsimd.tensor_mul(T4[:], T1[:], T1[:])
                nc.gpsimd.tensor_add(T3[:], T3[:], T4[:])
                nc.gpsimd.tensor_add(T4[:], T2[:], acc[:])
                nc.gpsimd.tensor_sub(T5[:], T4[:], T2[:])
                nc.gpsimd.tensor_sub(T0[:], T4[:], T5[:])
                nc.gpsimd.tensor_sub(T0[:], T2[:], T0[:])
                nc.gpsimd.tensor_sub(T1[:], acc[:], T5[:])
                nc.gpsimd.tensor_add(T0[:], T0[:], T1[:])
                nc.gpsimd.tensor_add(T0[:], T0[:], T3[:])
                nc.gpsimd.tensor_add(acc[:], T4[:], T0[:])
            nd2 = wpool.tile([128, NCAND], dt.float32, tag="nd2", name="nd2")
            nc.vector.tensor_scalar(nd2[:], acc[:], -1.0, None, op0=OP.mult)
            v8f = wpool.tile([128, 8], dt.float32, tag="v8f", name="v8f")
            nc.vector.max(out=v8f[:], in_=nd2[:])
            refb = wpool.tile([128, NCAND], dt.float32, tag="refb", name="refb")
            nc.vector.match_replace(out=refb[:], in_to_replace=v8f[:],
                                    in_values=nd2[:], imm_value=3e38)
            mask2 = wpool.tile([128, NCAND], dt.float32, tag="mask2", name="mask2")
            nc.vector.tensor_tensor(out=mask2[:], in0=refb[:], in1=nd2[:],
                                    op=OP.not_equal)
            dist = wpool.tile([128, NCAND], dt.float32, tag="dist16", name="dist16")
            nc.scalar.activation(dist[:], acc[:], AF.Sqrt)
            nc.vector.tensor_scalar_max(dist[:], dist[:], 1e-6)
            wr = wpool.tile([128, NCAND], dt.float32, tag="wr", name="wr")
            nc.vector.reciprocal(out=wr[:], in_=dist[:])
            nc.vector.tensor_mul(wr[:], wr[:], mask2[:])
            wsum = wpool.tile([128, 1], dt.float32, tag="wsum", name="wsum")
            nc.vector.tensor_reduce(out=wsum[:], in_=wr[:],
                                    axis=mybir.AxisListType.X, op=OP.add)
            wsr = wpool.tile([128, 1], dt.float32, tag="wsr", name="wsr")
            nc.vector.reciprocal(out=wsr[:], in_=wsum[:])
            wn = wpool.tile([128, NCAND], dt.float32, tag="wn", name="wn")
            nc.vector.tensor_scalar(wn[:], wr[:], wsr[:, 0:1], None, op0=OP.mult)

            # weighted mean of feats via DVE stt chain (node-major)
            facc = wpool.tile([128, C], dt.float32, tag="facc", name="facc")
            nc.vector.tensor_scalar(facc[:], gp[:, 0, 4:4 + C], wn[:, 0:1],
                                    None, op0=OP.mult)
            for k in range(1, NCAND):
                nc.vector.scalar_tensor_tensor(
                    out=facc[:], in0=gp[:, k, 4:4 + C], scalar=wn[:, k:k + 1],
                    in1=facc[:], op0=OP.mult, op1=OP.add)
            ps_fT = ppool.tile([128, C], dt.float32, tag="tr", name="psft")
            for half_i in range(2):
                nc.tensor.matmul(
                    ps_fT[:, half_i * 128:(half_i + 1) * 128],
                    lhsT=facc[:, half_i * 128:(half_i + 1) * 128],
                    rhs=eye_sb[:], is_transpose=True, start=True, stop=True)
            fT = wpool.tile([128, C], dt.float32, tag="fT", name="fT")
            nc.scalar.activation(fT[:], ps_fT[:], AF.Identity)

            def dense(src_sb, wpack, bias_sb, leaky, outtag):
                o = wpool.tile([128, C], dt.float32, tag=outtag, name=outtag)
                for d in range(2):
                    ps = ppool.tile([128, 128], dt.float32, tag="mm", name="psmm")
                    for ct in range(2):
                        nc.tensor.matmul(
                            ps[:], lhsT=wpack[:, (ct * 2 + d) * 128:(ct * 2 + d + 1) * 128],
                            rhs=src_sb[:, ct * 128:(ct + 1) * 128],
                            start=(ct == 0), stop=(ct == 1))
                    nc.scalar.activation(o[:, d * 128:(d + 1) * 128], ps[:],
                                         AF.Identity, bias=bias_sb[:, d:d + 1])
                if leaky:
                    tmp = wpool.tile([128, C], dt.float32, tag=outtag + "lk",
                                     name=outtag + "lk")
                    nc.vector.tensor_scalar(tmp[:], o[:], 0.1, None, op0=OP.mult)
                    nc.vector.tensor_max(o[:], o[:], tmp[:])
                return o

            mT = dense(fT, wp_sb, bproj_sb, False, "mT")
            h1T = dense(mT, wl1_sb, bl1_sb, True, "h1T")
            pT = dense(h1T, wl2_sb, fincol, False, "pT")
            osb = wpool.tile([128, C], dt.float32, tag="osb", name="osb")
            for dth in range(2):
                ps_tr = ppool.tile([128, 128], dt.float32, tag="mm", name="pstr")
                nc.tensor.matmul(ps_tr[:], lhsT=pT[:, dth * 128:(dth + 1) * 128],
                                 rhs=eye_sb[:], is_transpose=True,
                                 start=True, stop=True)
                nc.scalar.activation(osb[:, dth * 128:(dth + 1) * 128], ps_tr[:],
                                     AF.Identity)
            nc.sync.dma_start(out=out[t * 128:(t + 1) * 128, :], in_=osb[:])

        pending = None
        for t in range(NTILES + 1):
            if t < NTILES:
                st = stageA(t)
            if pending is not None:
                stageB(t - 1, pending)
            pending = st if t < NTILES else None

    nc.compile()
    _CACHE[key] = nc
    return nc


# ---------------------------------------------------------------- host entry
def kernel(node_coords, cond_coords, cond_feats, t,
           W_proj, b_proj, W_l1, b_l1, W_l2, b_l2, W_t1, b_t1, W_t2, b_t2):
    node_coords = np.asarray(node_coords)
    cond_coords = np.asarray(cond_coords)
    cond_feats = np.asarray(cond_feats, dtype=np.float32)
    mc = np.float32(node_coords.astype(np.float32).max())

    plan = _host_plan(node_coords, cond_coords)
    corder = plan['corder']
    norder = plan['norder']
    sel_blocks = plan['sel_blocks']
    budgets = plan['budgets']
    assign = plan['assign']          # [NTILES, NCORES]: global tile id
    pools = [b // 64 for b in budgets]

    part_c = _transform(cond_coords, 1.0, 0.01, mc)
    packed = np.ascontiguousarray(
        np.concatenate([part_c, cond_feats], 1)[corder])
    LHS, RHS = _split_rows(node_coords, cond_coords)
    RHSs = np.ascontiguousarray(RHS[:, corder])      # sorted cond order
    full_c = _transform(node_coords, 16.0, 0.05, mc)
    fill = _filler_col()

    eye = np.eye(128, dtype=f32)
    freqs1 = np.exp(np.arange(HALF, dtype=np.float32) *
                    f32(-math.log(10000.0) / (HALF - 1))).astype(f32)
    freqs = np.concatenate([freqs1, freqs1])
    shifts = np.concatenate([np.zeros(HALF, f32), np.full(HALF, PI / 2, f32)])

    nc = _build_program(budgets)
    in_maps = []
    node_order_core = []
    for ci in range(NCORES):
        tiles = [assign[k, ci] for k in range(NTILES)]
        nodes = np.concatenate([norder[g * 128:(g + 1) * 128] for g in tiles])
        node_order_core.append(nodes)
        # rhs window + invbase
        rhs_parts, invb_parts = [], []
        for k, g in enumerate(tiles):
            sel = sel_blocks[g]
            cols = np.concatenate([RHSs[:, b * BS:(b + 1) * BS] for b in sel], 1)
            nfill = budgets[k] - cols.shape[1]
            if nfill > 0:
                cols = np.concatenate(
                    [cols, np.tile(fill[:, None], (1, nfill))], 1)
            rhs_parts.append(cols)
            ib = np.empty(pools[k], f32)
            for gi in range(budgets[k] // BS):
                base = sel[gi] * BS if gi < len(sel) else 0
                ib[gi * 8:(gi + 1) * 8] = f32(32768.0 - base)
            invb_parts.append(ib)
        rhsw = np.ascontiguousarray(np.concatenate(rhs_parts, 1))
        invb = np.tile(np.concatenate(invb_parts)[None, :], (128, 1))
        nodex = np.ascontiguousarray(
            full_c[nodes, 1:4].reshape(NTILES, 128, 3).transpose(1, 0, 2)
            .reshape(128, NTILES * 3))
        in_maps.append({
            'lhsT': np.ascontiguousarray(LHS[:, nodes]),
            'rhsw': rhsw,
            'nodex': nodex,
            'packed': packed,
            'invbase': np.ascontiguousarray(invb),
            'eye': eye,
            'wp': _pack_w(np.asarray(W_proj, dtype=f32)),
            'wl1': _pack_w(np.asarray(W_l1, dtype=f32)),
            'wl2': _pack_w(np.asarray(W_l2, dtype=f32)),
            'bproj': np.asarray(b_proj, f32).reshape(2, 128).T.copy(),
            'bl1': np.asarray(b_l1, f32).reshape(2, 128).T.copy(),
            'bcomb': (np.asarray(b_l2, f32) + np.asarray(b_t2, f32)).reshape(2, 128).T.copy(),
            'wt1': np.ascontiguousarray(np.asarray(W_t1, f32).T),
            'wt2': np.ascontiguousarray(np.asarray(W_t2, f32).T),
            'bt1': np.asarray(b_t1, f32).reshape(EMBED, 1).copy(),
            'freqs': freqs.reshape(EMBED, 1).copy(),
            'shifts': shifts.reshape(EMBED, 1).copy(),
            'tval': np.full((EMBED, 1), np.asarray(t, f32).reshape(()), f32),
        })
    res = bass_utils.run_bass_kernel_spmd(nc, in_maps, core_ids=list(range(NCORES)))
    _CACHE['last_result'] = res
    out_full = np.empty((N, C), np.float32)
    for ci in range(NCORES):
        out_full[node_order_core[ci]] = res.results[ci]['out']
    return out_full


# revision 4
# speedup vs baseline: 2.1193x; 1.1117x over previous
"""Trainium2 Bass kernel for AttentiveMinkUNetDiff KNN+MLP block (v3).

Self-contained: hardcodes shapes N=16384, M=32768, K=8, C=256, 8 cores.

v3 strategy (windowed scan):
  Host: Morton-sort cond points; blocks of 512 consecutive sorted points.
  Nodes sorted by Morton of their box-clamped position; tiles of 128
  nodes. Per tile, select the cond blocks whose bbox is within a
  rigorous per-node footprint radius rr = sqrt(2*D*delta + delta^2)
  (delta from a density bound) of any member's clamped position -- this
  provably covers every node's kNN ball. Selected blocks are host-packed
  into a contiguous per-tile window (budget per slot, filler columns
  score strictly below every real point). Tiles are assigned to
  (core, slot) by descending window size so all cores share one
  per-slot budget schedule.

  Device per tile: 19-row exact bf16-split integer matmul over the
  window only; DVE MAX8/FIND_INDEX8 per 512-granule -> top-8-per-granule
  pool; top-12 candidates by value, reordered by ascending cond index;
  indirect-DMA gather of packed [coords|feats] rows; exact d^2 recompute
  (Dekker, matches XLA fma chain); final 8 by exact value; inverse-
  distance weights; weighted feature mean via DVE scalar_tensor_tensor
  chain; transpose; 3-layer MLP in transposed space; timestep branch
  folded into the final bias.
"""
import math
import numpy as np
import ml_dtypes

import concourse.bass as bass
import concourse.mybir as mybir
from concourse.tile import TileContext
from concourse import bass_utils
from concourse import bacc

bf16 = ml_dtypes.bfloat16
f32 = np.float32
AF = mybir.ActivationFunctionType
OP = mybir.AluOpType

N, M, K = 16384, 32768, 8
C = 256
PACK = C + 4                  # packed row: [part_c(4) | feats(256)]
EMBED, HALF = 96, 48
NCORES = 8
NSHARD = N // NCORES          # 2048
NTILES = NSHARD // 128        # 16
BS = 512                      # cond block / scan granule width
NB = M // BS                  # 64 blocks
CSAFE = 40.0
NCAND = 12
PI = float(np.pi)
NEG = -3.0e38


# ---------------------------------------------------------------- host prep
def _split_rows(nodes, conds):
    """Build the 19 bf16-exact contraction rows. Validated vs reference."""
    a = nodes[:, 1:4].astype(np.int64)
    b = conds[:, 1:4].astype(np.int64)
    ah, al = a >> 5, a & 31
    bh, bl = b >> 5, b & 31
    lhs, rhs = [], []
    for k in range(3):
        lhs += [1280.0 * ah[:, k], 1280.0 * ah[:, k], 40.0 * al[:, k], 40.0 * al[:, k]]
        rhs += [32.0 * bh[:, k], 1.0 * bl[:, k], 32.0 * bh[:, k], 1.0 * bl[:, k]]
    B_total = (4 * b * b - 316 * b).sum(1) + 32768
    s2, s1, s0 = B_total >> 16, (B_total >> 8) & 255, B_total & 255
    nones = -np.ones(a.shape[0])
    lhs += [nones, nones, nones]
    rhs += [s2 * 65536.0, s1 * 256.0, s0 * 1.0]
    C_i = ((10 * a + 79) ** 2).sum(1) - 32768
    c3 = np.floor(C_i / 2 ** 21).astype(np.int64)
    r = C_i - c3 * 2 ** 21
    c2, c1, c0 = r >> 13, (r >> 5) & 255, r & 31
    mones = np.ones(b.shape[0])
    lhs += [-c3 * 2097152.0, -c2 * 8192.0, -c1 * 32.0, -c0 * 1.0]
    rhs += [mones, mones, mones, mones]
    LHS = np.stack(lhs).astype(f32)   # [19, N]
    RHS = np.stack(rhs).astype(f32)   # [19, M]
    return LHS.astype(bf16), RHS.astype(bf16)


def _filler_col():
    v = np.zeros(19, f32)
    v[12], v[13], v[14] = 255.0 * 65536.0, 255.0 * 256.0, 255.0
    v[15:19] = 1.0
    return v.astype(bf16)


def _transform(coords, stride, voxel, mc):
    c = coords.astype(np.float32)
    batch = (c[:, :1] * f32(mc * f32(2.0))).astype(f32)
    xyz = ((c[:, 1:] + f32(stride / 2.0)).astype(f32) * f32(voxel)).astype(f32)
    return np.concatenate([batch, xyz], 1).astype(f32)


def _pack_w(w):
    """W [dout, din] -> lhsT pack [128, 4*128]: col block (ct*2+dt)."""
    wt = np.ascontiguousarray(w.T.astype(f32))          # [din, dout]
    p = wt.reshape(2, 128, 2, 128)                      # [ct, c, dt, d]
    p = p.transpose(1, 0, 2, 3).reshape(128, 512)
    return np.ascontiguousarray(p)


def _morton3(u):
    u = u.astype(np.uint64)

    def spread(v):
        v = v & 0x3ff
        v = (v | (v << 16)) & 0x030000FF
        v = (v | (v << 8)) & 0x0300F00F
        v = (v | (v << 4)) & 0x030C30C3
        v = (v | (v << 2)) & 0x09249249
        return v
    return (spread(u[:, 0]) << 2) | (spread(u[:, 1]) << 1) | spread(u[:, 2])


def _host_plan(node_coords, cond_coords):
    """Sort orders, per-tile block selection, slot schedule, assignment."""
    nx = (node_coords[:, 1:4].astype(np.float32) + f32(8.0)) * f32(0.05)
    cx = (cond_coords[:, 1:4].astype(np.float32) + f32(0.5)) * f32(0.01)
    ck = _morton3(cond_coords[:, 1:4])
    corder = np.argsort(ck, kind='stable')
    bmin, bmax = cx.min(0), cx.max(0)
    clamped = np.clip(nx, bmin, bmax)
    D = np.linalg.norm(nx - clamped, axis=1)
    dens = M / float(np.prod((bmax - bmin).astype(np.float64)))
    outdims = ((nx < bmin) | (nx > bmax)).sum(1)
    ceff = CSAFE * np.maximum(1.0, 2.0 ** (outdims - 1))
    vol = ceff / dens
    delta_face = np.sqrt(vol / (np.pi * np.maximum(D, 1e-9)))
    delta_sph = (vol * 3.0 / (4.0 * np.pi)) ** (1.0 / 3.0)
    delta = np.where(D > 0.3, delta_face, delta_sph)
    rr = np.sqrt(2 * D * delta + delta ** 2)

    ngrid = np.clip(np.round(clamped / 0.01 - 0.5), 0, 1023).astype(np.int64)
    nk = _morton3(ngrid)
    norder = np.argsort(nk, kind='stable')

    bidx = corder.reshape(NB, BS)
    bco = cx[bidx]
    blo, bhi = bco.min(1), bco.max(1)

    ntile = N // 128
    sel_blocks, w_tiles = [], []
    for t in range(ntile):
        mem = norder[t * 128:(t + 1) * 128]
        s, r = clamped[mem], rr[mem]
        d = (np.maximum(blo[None] - s[:, None], 0)
             + np.maximum(s[:, None] - bhi[None], 0))
        need = (np.linalg.norm(d, axis=2) <= r[:, None]).any(0)
        sel = np.where(need)[0]
        sel_blocks.append(sel)
        w_tiles.append(len(sel) * BS)
    w_tiles = np.array(w_tiles)
    pi = np.argsort(-w_tiles, kind='stable')      # tiles sorted by W desc
    budgets = tuple(int(w_tiles[pi[8 * k]]) for k in range(NTILES))
    # tile pi[8k+c] -> core c, slot k
    assign = pi.reshape(NTILES, NCORES)
    return dict(corder=corder, norder=norder, sel_blocks=sel_blocks,
                budgets=budgets, assign=assign)


_CACHE = {}


def _build_program(budgets):
    key = ('nc', budgets)
    if key in _CACHE:
        return _CACHE[key]
    nc = bacc.Bacc("TRN2", target_bir_lowering=False, debug=False,
                   num_devices=NCORES)
    dt = mybir.dt
    WMAX = max(budgets)
    WTOT = sum(budgets)
    pools = [b // 64 for b in budgets]             # pool width per slot
    PTOT = sum(pools)
    PMAX = max(pools)
    woff = np.cumsum([0] + [b for b in budgets])
    poff = np.cumsum([0] + pools)

    def din(name, shape, dtype):
        return nc.dram_tensor(name, shape, dtype, kind="ExternalInput").ap()

    lhsT = din('lhsT', [19, NSHARD], dt.bfloat16)
    rhsw = din('rhsw', [19, WTOT], dt.bfloat16)
    nodex = din('nodex', [128, NTILES * 3], dt.float32)
    packed = din('packed', [M, PACK], dt.float32)
    invbase = din('invbase', [128, PTOT], dt.float32)
    eye = din('eye', [128, 128], dt.float32)
    wp = din('wp', [128, 512], dt.float32)
    wl1 = din('wl1', [128, 512], dt.float32)
    wl2 = din('wl2', [128, 512], dt.float32)
    bproj = din('bproj', [128, 2], dt.float32)
    bl1 = din('bl1', [128, 2], dt.float32)
    bcomb = din('bcomb', [128, 2], dt.float32)
    wt1 = din('wt1', [EMBED, EMBED], dt.float32)
    wt2 = din('wt2', [EMBED, C], dt.float32)
    bt1 = din('bt1', [EMBED, 1], dt.float32)
    freqs = din('freqs', [EMBED, 1], dt.float32)
    shifts = din('shifts', [EMBED, 1], dt.float32)
    tval = din('tval', [EMBED, 1], dt.float32)
    out = nc.dram_tensor('out', [NSHARD, C], dt.float32, kind="ExternalOutput").ap()

    with TileContext(nc) as tc, \
            tc.tile_pool(name="const", bufs=1) as cpool, \
            tc.tile_pool(name="work", bufs=2) as wpool, \
            tc.tile_pool(name="psum", bufs=2, space="PSUM") as ppool:

        # ---- constants to SBUF
        lhs_sb = cpool.tile([19, NSHARD], dt.bfloat16, tag="lhs")
        nc.sync.dma_start(out=lhs_sb[:], in_=lhsT)
        nodex_sb = cpool.tile([128, NTILES * 3], dt.float32, tag="nodex")
        nc.sync.dma_start(out=nodex_sb[:], in_=nodex)
        invb_sb = cpool.tile([128, PTOT], dt.float32, tag="invb")
        nc.sync.dma_start(out=invb_sb[:], in_=invbase)
        eye_sb = cpool.tile([128, 128], dt.float32, tag="eye")
        nc.sync.dma_start(out=eye_sb[:], in_=eye)
        wp_sb = cpool.tile([128, 512], dt.float32, tag="wp")
        nc.sync.dma_start(out=wp_sb[:], in_=wp)
        wl1_sb = cpool.tile([128, 512], dt.float32, tag="wl1")
        nc.sync.dma_start(out=wl1_sb[:], in_=wl1)
        wl2_sb = cpool.tile([128, 512], dt.float32, tag="wl2")
        nc.sync.dma_start(out=wl2_sb[:], in_=wl2)
        bproj_sb = cpool.tile([128, 2], dt.float32, tag="bproj")
        nc.sync.dma_start(out=bproj_sb[:], in_=bproj)
        bl1_sb = cpool.tile([128, 2], dt.float32, tag="bl1")
        nc.sync.dma_start(out=bl1_sb[:], in_=bl1)
        bcomb_sb = cpool.tile([128, 2], dt.float32, tag="bcomb")
        nc.sync.dma_start(out=bcomb_sb[:], in_=bcomb)
        wt1_sb = cpool.tile([EMBED, EMBED], dt.float32, tag="wt1")
        nc.sync.dma_start(out=wt1_sb[:], in_=wt1)
        wt2_sb = cpool.tile([EMBED, C], dt.float32, tag="wt2")
        nc.sync.dma_start(out=wt2_sb[:], in_=wt2)
        bt1_sb = cpool.tile([EMBED, 1], dt.float32, tag="bt1")
        nc.sync.dma_start(out=bt1_sb[:], in_=bt1)
        fr_sb = cpool.tile([EMBED, 1], dt.float32, tag="fr")
        nc.sync.dma_start(out=fr_sb[:], in_=freqs)
        sh_sb = cpool.tile([EMBED, 1], dt.float32, tag="sh")
        nc.sync.dma_start(out=sh_sb[:], in_=shifts)
        t_sb = cpool.tile([EMBED, 1], dt.float32, tag="t1x1")
        nc.sync.dma_start(out=t_sb[:], in_=tval)

        # ---- t branch -> fincol [128, 2]
        e = cpool.tile([EMBED, 1], dt.float32, tag="e")
        nc.vector.tensor_mul(e[:], t_sb[:], fr_sb[:])
        nc.vector.tensor_add(e[:], e[:], sh_sb[:])
        ki = cpool.tile([EMBED, 1], dt.int32, tag="ki")
        kf = cpool.tile([EMBED, 1], dt.float32, tag="kf")
        nc.vector.tensor_scalar(kf[:], e[:], float(1.0 / (2 * PI)), None, op0=OP.mult)
        nc.vector.tensor_copy(out=ki[:], in_=kf[:])
        nc.vector.tensor_copy(out=kf[:], in_=ki[:])
        nc.vector.tensor_scalar(kf[:], kf[:], float(2 * PI), None, op0=OP.mult)
        nc.vector.tensor_sub(e[:], e[:], kf[:])
        gt = cpool.tile([EMBED, 1], dt.float32, tag="gt")
        nc.vector.tensor_scalar(gt[:], e[:], float(PI), None, op0=OP.is_gt)
        nc.vector.tensor_scalar(gt[:], gt[:], float(2 * PI), None, op0=OP.mult)
        nc.vector.tensor_sub(e[:], e[:], gt[:])
        emb_sb = cpool.tile([EMBED, 1], dt.float32, tag="emb")
        nc.scalar.activation(emb_sb[:], e[:], AF.Sin)
        ps_t1 = ppool.tile([EMBED, 1], dt.float32, tag="mm")
        nc.tensor.matmul(ps_t1[:], lhsT=wt1_sb[:], rhs=emb_sb[:], start=True, stop=True)
        h96 = cpool.tile([EMBED, 1], dt.float32, tag="h96")
        nc.scalar.activation(h96[:], ps_t1[:], AF.Identity, bias=bt1_sb[:, 0:1])
        h96b = cpool.tile([EMBED, 1], dt.float32, tag="h96b")
        nc.vector.tensor_scalar(h96b[:], h96[:], 0.1, None, op0=OP.mult)
        nc.vector.tensor_max(h96b[:], h96b[:], h96[:])
        fincol = cpool.tile([128, 2], dt.float32, tag="fincol")
        for d in range(2):
            ps_t2 = ppool.tile([128, 1], dt.float32, tag="mm")
            nc.tensor.matmul(ps_t2[:], lhsT=wt2_sb[:, d * 128:(d + 1) * 128],
                             rhs=h96b[:], start=True, stop=True)
            nc.scalar.activation(fincol[:, d:d + 1], ps_t2[:], AF.Identity,
                                 bias=bcomb_sb[:, d:d + 1])

        c32k = cpool.tile([128, NCAND], dt.float32, tag="c32k")
        nc.vector.memset(c32k[:], 32768.0)

        # ---- main loop, software-pipelined: A(t+1) emitted before B(t)
        def stageA(t):
            st = {}
            Wk = budgets[t]
            Gk = Wk // BS
            Pk = pools[t]
            lt = lhs_sb[:, t * 128:(t + 1) * 128]
            rhs_sb = wpool.tile([19, WMAX], dt.bfloat16, tag="rhsw", name="rhsw",
                                bufs=2)
            for j in range(0, Wk, 4096):
                w = min(4096, Wk - j)
                nc.sync.dma_start(out=rhs_sb[:, j:j + w],
                                  in_=rhsw[:, woff[t] + j:woff[t] + j + w])
            pool_vals = wpool.tile([128, PMAX], dt.float32, tag="pvals",
                                   name="pvals", bufs=2)
            pool_lidx = wpool.tile([128, PMAX], dt.uint16, tag="plidx",
                                   name="plidx", bufs=2)
            for g in range(Gk):
                ps_d = ppool.tile([128, BS], dt.float32, tag="dist", name="psd",
                                  bufs=4)
                nc.tensor.matmul(ps_d[:], lhsT=lt,
                                 rhs=rhs_sb[:, g * BS:(g + 1) * BS],
                                 start=True, stop=True)
                rowb = wpool.tile([128, BS], dt.float32, tag="rowb", name="rowb",
                                  bufs=6)
                nc.scalar.activation(rowb[:], ps_d[:], AF.Identity)
                nc.vector.max(out=pool_vals[:, g * 8:(g + 1) * 8], in_=rowb[:])
                nc.vector.max_index(out=pool_lidx[:, g * 8:(g + 1) * 8],
                                    in_max=pool_vals[:, g * 8:(g + 1) * 8],
                                    in_values=rowb[:])
            pv = pool_vals[:, 0:Pk]
            lidxf = wpool.tile([128, PMAX], dt.float32, tag="lidxf", name="lidxf")
            nc.vector.tensor_copy(out=lidxf[:, 0:Pk], in_=pool_lidx[:, 0:Pk])
            pinv = wpool.tile([128, PMAX], dt.float32, tag="pinv", name="pinv")
            nc.vector.tensor_sub(pinv[:, 0:Pk],
                                 invb_sb[:, poff[t]:poff[t] + Pk],
                                 lidxf[:, 0:Pk])
            # top-12 by value: v8 (ranks 1-8), v8b[:,0:4] (ranks 9-12)
            v8 = wpool.tile([128, 8], dt.float32, tag="v8", name="v8")
            pv2 = wpool.tile([128, PMAX], dt.float32, tag="pv2", name="pv2")
            pv3 = wpool.tile([128, PMAX], dt.float32, tag="pv3", name="pv3")
            nc.vector.max(out=v8[:], in_=pv)
            nc.vector.match_replace(out=pv2[:, 0:Pk], in_to_replace=v8[:],
                                    in_values=pv, imm_value=NEG)
            v8b = wpool.tile([128, 8], dt.float32, tag="v8b", name="v8b")
            nc.vector.max(out=v8b[:], in_=pv2[:, 0:Pk])
            v12p = wpool.tile([128, 8], dt.float32, tag="v12p", name="v12p")
            nc.vector.memset(v12p[:], NEG)
            nc.vector.tensor_copy(out=v12p[:, 0:4], in_=v8b[:, 0:4])
            nc.vector.match_replace(out=pv3[:, 0:Pk], in_to_replace=v12p[:],
                                    in_values=pv2[:, 0:Pk], imm_value=NEG)
            maskp = wpool.tile([128, PMAX], dt.float32, tag="maskp", name="maskp")
            nc.vector.tensor_tensor(out=maskp[:, 0:Pk], in0=pv3[:, 0:Pk],
                                    in1=pv, op=OP.not_equal)
            nc.vector.tensor_mul(maskp[:, 0:Pk], maskp[:, 0:Pk], pinv[:, 0:Pk])
            # order the 12 by ascending global index
            i8a = wpool.tile([128, 8], dt.float32, tag="i8a", name="i8a")
            nc.vector.max(out=i8a[:], in_=maskp[:, 0:Pk])
            mv2 = wpool.tile([128, PMAX], dt.float32, tag="mv2", name="mv2")
            nc.vector.match_replace(out=mv2[:, 0:Pk], in_to_replace=i8a[:],
                                    in_values=maskp[:, 0:Pk], imm_value=0.0)
            i8b = wpool.tile([128, 8], dt.float32, tag="i8b", name="i8b")
            nc.vector.max(out=i8b[:], in_=mv2[:, 0:Pk])
            inv12 = wpool.tile([128, NCAND], dt.float32, tag="inv12", name="inv12")
            nc.vector.tensor_copy(out=inv12[:, 0:8], in_=i8a[:])
            nc.vector.tensor_copy(out=inv12[:, 8:12], in_=i8b[:, 0:4])
            g12f = wpool.tile([128, NCAND], dt.float32, tag="g12f", name="g12f")
            nc.vector.tensor_sub(g12f[:], c32k[:], inv12[:])
            g12u = wpool.tile([128, NCAND], dt.uint32, tag="g12u", name="g12u")
            nc.vector.tensor_copy(out=g12u[:], in_=g12f[:])
            gp = wpool.tile([128, NCAND, PACK], dt.float32, tag="gp", name="gp",
                            bufs=4)
            for k in range(NCAND):
                nc.gpsimd.indirect_dma_start(
                    out=gp[:, k, :], out_offset=None, in_=packed,
                    in_offset=bass.IndirectOffsetOnAxis(ap=g12u[:, k:k + 1], axis=0))
            st['gp'] = gp
            return st

        def stageB(t, st):
            gp = st['gp']
            xs = [nodex_sb[:, t * 3 + k: t * 3 + k + 1] for k in range(3)]
            dcol = [wpool.tile([128, NCAND], dt.float32, tag=f"d{k}",
                               name=f"dcol{k}") for k in range(3)]
            for k in range(3):
                nc.vector.tensor_scalar(dcol[k][:], gp[:, :, k + 1], xs[k],
                                        None, op0=OP.subtract)
            acc = wpool.tile([128, NCAND], dt.float32, tag="acc", name="acc")
            nc.gpsimd.tensor_mul(acc[:], dcol[0][:], dcol[0][:])
            tt = [wpool.tile([128, NCAND], dt.float32, tag=f"t{i}",
                             name=f"tt{i}") for i in range(6)]
            for k in (1, 2):
                d = dcol[k]
                T0, T1, T2, T3, T4, T5 = tt
                nc.vector.tensor_scalar(T0[:], d[:], 4097.0, None, op0=OP.mult)
                nc.gpsimd.tensor_sub(T1[:], T0[:], d[:])
                nc.gpsimd.tensor_sub(T0[:], T0[:], T1[:])
                nc.gpsimd.tensor_sub(T1[:], d[:], T0[:])
                nc.gpsimd.tensor_mul(T2[:], d[:], d[:])
                nc.gpsimd.tensor_mul(T3[:], T0[:], T0[:])
                nc.gpsimd.tensor_sub(T3[:], T3[:], T2[:])
                nc.gpsimd.tensor_add(T4[:], T1[:], T1[:])
                nc.gpsimd.tensor_mul(T4[:], T0[:], T4[:])
                nc.gpsimd.tensor_add(T3[:], T3[:], T4[:])
                nc.gpsimd.tensor_mul(T4[:], T1[:], T1[:])
                nc.gpsimd.tensor_add(T3[:], T3[:], T4[:])
                nc.gpsimd.tensor_add(T4[:], T2[:], acc[:])
                nc.gpsimd.tensor_sub(T5[:], T4[:], T2[:])
                nc.gpsimd.tensor_sub(T0[:], T4[:], T5[:])
                nc.gpsimd.tensor_sub(T0[:], T2[:], T0[:])
                nc.gpsimd.tensor_sub(T1[:], acc[:], T5[:])
                nc.gpsimd.tensor_add(T0[:], T0[:], T1[:])
                nc.gpsimd.tensor_add(T0[:], T0[:], T3[:])
                nc.gpsimd.tensor_add(acc[:], T4[:], T0[:])
            nd2 = wpool.tile([128, NCAND], dt.float32, tag="nd2", name="nd2")
            nc.vector.tensor_scalar(nd2[:], acc[:], -1.0, None, op0=OP.mult)
            v8f = wpool.tile([128, 8], dt.float32, tag="v8f", name="v8f")
            nc.vector.max(out=v8f[:], in_=nd2[:])
            refb = wpool.tile([128, NCAND], dt.float32, tag="refb", name="refb")
            nc.vector.match_replace(out=refb[:], in_to_replace=v8f[:],
                                    in_values=nd2[:], imm_value=3e38)
            mask2 = wpool.tile([128, NCAND], dt.float32, tag="mask2", name="mask2")
            nc.vector.tensor_tensor(out=mask2[:], in0=refb[:], in1=nd2[:],
                                    op=OP.not_equal)
            dist = wpool.tile([128, NCAND], dt.float32, tag="dist16", name="dist16")
            nc.scalar.activation(dist[:], acc[:], AF.Sqrt)
            nc.vector.tensor_scalar_max(dist[:], dist[:], 1e-6)
            wr = wpool.tile([128, NCAND], dt.float32, tag="wr", name="wr")
            nc.vector.reciprocal(out=wr[:], in_=dist[:])
            nc.vector.tensor_mul(wr[:], wr[:], mask2[:])
            wsum = wpool.tile([128, 1], dt.float32, tag="wsum", name="wsum")
            nc.vector.tensor_reduce(out=wsum[:], in_=wr[:],
                                    axis=mybir.AxisListType.X, op=OP.add)
            wsr = wpool.tile([128, 1], dt.float32, tag="wsr", name="wsr")
            nc.vector.reciprocal(out=wsr[:], in_=wsum[:])
            wn = wpool.tile([128, NCAND], dt.float32, tag="wn", name="wn")
            nc.vector.tensor_scalar(wn[:], wr[:], wsr[:, 0:1], None, op0=OP.mult)

            # weighted mean of feats via DVE stt chain (node-major)
            facc = wpool.tile([128, C], dt.float32, tag="facc", name="facc")
            nc.vector.tensor_scalar(facc[:], gp[:, 0, 4:4 + C], wn[:, 0:1],
                                    None, op0=OP.mult)
            for k in range(1, NCAND):
                nc.vector.scalar_tensor_tensor(
                    out=facc[:], in0=gp[:, k, 4:4 + C], scalar=wn[:, k:k + 1],
                    in1=facc[:], op0=OP.mult, op1=OP.add)
            ps_fT = ppool.tile([128, C], dt.float32, tag="tr", name="psft")
            for half_i in range(2):
                nc.tensor.matmul(
                    ps_fT[:, half_i * 128:(half_i + 1) * 128],
                    lhsT=facc[:, half_i * 128:(half_i + 1) * 128],
                    rhs=eye_sb[:], is_transpose=True, start=True, stop=True)
            fT = wpool.tile([128, C], dt.float32, tag="fT", name="fT")
            nc.scalar.activation(fT[:], ps_fT[:], AF.Identity)

            def dense(src_sb, wpack, bias_sb, leaky, outtag):
                o = wpool.tile([128, C], dt.float32, tag=outtag, name=outtag)
                for d in range(2):
                    ps = ppool.tile([128, 128], dt.float32, tag="mm", name="psmm")
                    for ct in range(2):
                        nc.tensor.matmul(
                            ps[:], lhsT=wpack[:, (ct * 2 + d) * 128:(ct * 2 + d + 1) * 128],
                            rhs=src_sb[:, ct * 128:(ct + 1) * 128],
                            start=(ct == 0), stop=(ct == 1))
                    nc.scalar.activation(o[:, d * 128:(d + 1) * 128], ps[:],
                                         AF.Identity, bias=bias_sb[:, d:d + 1])
                if leaky:
                    tmp = wpool.tile([128, C], dt.float32, tag=outtag + "lk",
                                     name=outtag + "lk")
                    nc.vector.tensor_scalar(tmp[:], o[:], 0.1, None, op0=OP.mult)
                    nc.vector.tensor_max(o[:], o[:], tmp[:])
                return o

            mT = dense(fT, wp_sb, bproj_sb, False, "mT")
            h1T = dense(mT, wl1_sb, bl1_sb, True, "h1T")
            pT = dense(h1T, wl2_sb, fincol, False, "pT")
            osb = wpool.tile([128, C], dt.float32, tag="osb", name="osb")
            for dth in range(2):
                ps_tr = ppool.tile([128, 128], dt.float32, tag="mm", name="pstr")
                nc.tensor.matmul(ps_tr[:], lhsT=pT[:, dth * 128:(dth + 1) * 128],
                                 rhs=eye_sb[:], is_transpose=True,
                                 start=True, stop=True)
                nc.scalar.activation(osb[:, dth * 128:(dth + 1) * 128], ps_tr[:],
                                     AF.Identity)
            nc.sync.dma_start(out=out[t * 128:(t + 1) * 128, :], in_=osb[:])

        pend = []
        for t in range(NTILES + 2):
            if t < NTILES:
                pend.append(stageA(t))
            if t >= 2:
                stageB(t - 2, pend.pop(0))

    nc.compile()
    _CACHE[key] = nc
    return nc


# ---------------------------------------------------------------- host entry
def kernel(node_coords, cond_coords, cond_feats, t,
           W_proj, b_proj, W_l1, b_l1, W_l2, b_l2, W_t1, b_t1, W_t2, b_t2):
    node_coords = np.asarray(node_coords)
    cond_coords = np.asarray(cond_coords)
    cond_feats = np.asarray(cond_feats, dtype=np.float32)
    mc = np.float32(node_coords.astype(np.float32).max())

    plan = _host_plan(node_coords, cond_coords)
    corder = plan['corder']
    norder = plan['norder']
    sel_blocks = plan['sel_blocks']
    budgets = plan['budgets']
    assign = plan['assign']          # [NTILES, NCORES]: global tile id
    pools = [b // 64 for b in budgets]

    part_c = _transform(cond_coords, 1.0, 0.01, mc)
    packed = np.ascontiguousarray(
        np.concatenate([part_c, cond_feats], 1)[corder])
    LHS, RHS = _split_rows(node_coords, cond_coords)
    RHSs = np.ascontiguousarray(RHS[:, corder])      # sorted cond order
    full_c = _transform(node_coords, 16.0, 0.05, mc)
    fill = _filler_col()

    eye = np.eye(128, dtype=f32)
    freqs1 = np.exp(np.arange(HALF, dtype=np.float32) *
                    f32(-math.log(10000.0) / (HALF - 1))).astype(f32)
    freqs = np.concatenate([freqs1, freqs1])
    shifts = np.concatenate([np.zeros(HALF, f32), np.full(HALF, PI / 2, f32)])

    nc = _build_program(budgets)
    in_maps = []
    node_order_core = []
    for ci in range(NCORES):
        tiles = [assign[k, ci] for k in range(NTILES)]
        nodes = np.concatenate([norder[g * 128:(g + 1) * 128] for g in tiles])
        node_order_core.append(nodes)
        # rhs window + invbase
        rhs_parts, invb_parts = [], []
        for k, g in enumerate(tiles):
            sel = sel_blocks[g]
            cols = np.concatenate([RHSs[:, b * BS:(b + 1) * BS] for b in sel], 1)
            nfill = budgets[k] - cols.shape[1]
            if nfill > 0:
                cols = np.concatenate(
                    [cols, np.tile(fill[:, None], (1, nfill))], 1)
            rhs_parts.append(cols)
            ib = np.empty(pools[k], f32)
            for gi in range(budgets[k] // BS):
                base = sel[gi] * BS if gi < len(sel) else 0
                ib[gi * 8:(gi + 1) * 8] = f32(32768.0 - base)
            invb_parts.append(ib)
        rhsw = np.ascontiguousarray(np.concatenate(rhs_parts, 1))
        invb = np.tile(np.concatenate(invb_parts)[None, :], (128, 1))
        nodex = np.ascontiguousarray(
            full_c[nodes, 1:4].reshape(NTILES, 128, 3).transpose(1, 0, 2)
            .reshape(128, NTILES * 3))
        in_maps.append({
            'lhsT': np.ascontiguousarray(LHS[:, nodes]),
            'rhsw': rhsw,
            'nodex': nodex,
            'packed': packed,
            'invbase': np.ascontiguousarray(invb),
            'eye': eye,
            'wp': _pack_w(np.asarray(W_proj, dtype=f32)),
            'wl1': _pack_w(np.asarray(W_l1, dtype=f32)),
            'wl2': _pack_w(np.asarray(W_l2, dtype=f32)),
            'bproj': np.asarray(b_proj, f32).reshape(2, 128).T.copy(),
            'bl1': np.asarray(b_l1, f32).reshape(2, 128).T.copy(),
            'bcomb': (np.asarray(b_l2, f32) + np.asarray(b_t2, f32)).reshape(2, 128).T.copy(),
            'wt1': np.ascontiguousarray(np.asarray(W_t1, f32).T),
            'wt2': np.ascontiguousarray(np.asarray(W_t2, f32).T),
            'bt1': np.asarray(b_t1, f32).reshape(EMBED, 1).copy(),
            'freqs': freqs.reshape(EMBED, 1).copy(),
            'shifts': shifts.reshape(EMBED, 1).copy(),
            'tval': np.full((EMBED, 1), np.asarray(t, f32).reshape(()), f32),
        })
    res = bass_utils.run_bass_kernel_spmd(nc, in_maps, core_ids=list(range(NCORES)))
    _CACHE['last_result'] = res
    out_full = np.empty((N, C), np.float32)
    for ci in range(NCORES):
        out_full[node_order_core[ci]] = res.results[ci]['out']
    return out_full


# revision 7
# speedup vs baseline: 2.3102x; 1.0901x over previous
"""Trainium2 Bass kernel for AttentiveMinkUNetDiff KNN+MLP block (v4).

Self-contained: hardcodes shapes N=16384, M=32768, K=8, C=256, 8 cores.

Strategy (windowed scan):
  Host: Morton-sort cond points; blocks of 512 consecutive sorted points.
  Nodes sorted by Morton of their box-clamped position; tiles of 128
  nodes. Per tile, select the cond blocks whose bbox is within a
  rigorous per-node footprint radius rr = sqrt(2*D*delta + delta^2)
  (delta from a density bound) of any member's clamped position -- this
  provably covers every node's kNN ball. Selected blocks are host-packed
  into a contiguous per-tile window (fixed per-slot budget schedule,
  filler columns score strictly below every real point). Tiles are
  assigned to (core, slot) by descending window size so all cores share
  one schedule.

  Device per tile: 19-row exact bf16-split integer matmul over the
  window; DVE MAX8/FIND_INDEX8 per 1024-granule (= block pair; index
  recovered piecewise-affine) -> top-8-per-granule pool; top-10
  candidates by value, reordered by ascending cond index; batched
  indirect-DMA gather of packed [coords|feats] rows; exact d^2
  recompute (Dekker, matches XLA fma chain); final 8 by exact value;
  inverse-distance weights; weighted feature mean via DVE
  scalar_tensor_tensor chain; transpose; 3-layer MLP batched over 4
  tiles in transposed space; timestep branch folded into final bias.
"""
import math
import numpy as np
import ml_dtypes

import concourse.bass as bass
import concourse.mybir as mybir
from concourse.tile import TileContext
from concourse import bass_utils
from concourse import bacc

bf16 = ml_dtypes.bfloat16
f32 = np.float32
AF = mybir.ActivationFunctionType
OP = mybir.AluOpType

N, M, K = 16384, 32768, 8
C = 256
PACK = C + 4                  # packed row: [part_c(4) | feats(256)]
EMBED, HALF = 96, 48
NCORES = 8
NSHARD = N // NCORES          # 2048
NTILES = NSHARD // 128        # 16
BS = 512                      # cond block width
GR = 1024                     # scan granule (block pair)
NB = M // BS                  # 64 blocks
CSAFE = 40.0
NCAND = 10
PI = float(np.pi)
NEG = -3.0e38


# ---------------------------------------------------------------- host prep
def _split_rows(nodes, conds):
    """Build the 19 bf16-exact contraction rows. Validated vs reference."""
    a = nodes[:, 1:4].astype(np.int64)
    b = conds[:, 1:4].astype(np.int64)
    ah, al = a >> 5, a & 31
    bh, bl = b >> 5, b & 31
    lhs, rhs = [], []
    for k in range(3):
        lhs += [1280.0 * ah[:, k], 1280.0 * ah[:, k], 40.0 * al[:, k], 40.0 * al[:, k]]
        rhs += [32.0 * bh[:, k], 1.0 * bl[:, k], 32.0 * bh[:, k], 1.0 * bl[:, k]]
    B_total = (4 * b * b - 316 * b).sum(1) + 32768
    s2, s1, s0 = B_total >> 16, (B_total >> 8) & 255, B_total & 255
    nones = -np.ones(a.shape[0])
    lhs += [nones, nones, nones]
    rhs += [s2 * 65536.0, s1 * 256.0, s0 * 1.0]
    C_i = ((10 * a + 79) ** 2).sum(1) - 32768
    c3 = np.floor(C_i / 2 ** 21).astype(np.int64)
    r = C_i - c3 * 2 ** 21
    c2, c1, c0 = r >> 13, (r >> 5) & 255, r & 31
    mones = np.ones(b.shape[0])
    lhs += [-c3 * 2097152.0, -c2 * 8192.0, -c1 * 32.0, -c0 * 1.0]
    rhs += [mones, mones, mones, mones]
    LHS = np.stack(lhs).astype(f32)   # [19, N]
    RHS = np.stack(rhs).astype(f32)   # [19, M]
    return LHS.astype(bf16), RHS.astype(bf16)


def _filler_col():
    v = np.zeros(19, f32)
    v[12], v[13], v[14] = 255.0 * 65536.0, 255.0 * 256.0, 255.0
    v[15:19] = 1.0
    return v.astype(bf16)


def _transform(coords, stride, voxel, mc):
    c = coords.astype(np.float32)
    batch = (c[:, :1] * f32(mc * f32(2.0))).astype(f32)
    xyz = ((c[:, 1:] + f32(stride / 2.0)).astype(f32) * f32(voxel)).astype(f32)
    return np.concatenate([batch, xyz], 1).astype(f32)


def _pack_w(w):
    """W [dout, din] -> lhsT pack [128, 4*128]: col block (ct*2+dt)."""
    wt = np.ascontiguousarray(w.T.astype(f32))          # [din, dout]
    p = wt.reshape(2, 128, 2, 128)                      # [ct, c, dt, d]
    p = p.transpose(1, 0, 2, 3).reshape(128, 512)
    return np.ascontiguousarray(p)


def _morton3(u):
    u = u.astype(np.uint64)

    def spread(v):
        v = v & 0x3ff
        v = (v | (v << 16)) & 0x030000FF
        v = (v | (v << 8)) & 0x0300F00F
        v = (v | (v << 4)) & 0x030C30C3
        v = (v | (v << 2)) & 0x09249249
        return v
    return (spread(u[:, 0]) << 2) | (spread(u[:, 1]) << 1) | spread(u[:, 2])


def _host_plan(node_coords, cond_coords):
    """Sort orders, per-tile block selection, slot schedule, assignment."""
    nx = (node_coords[:, 1:4].astype(np.float32) + f32(8.0)) * f32(0.05)
    cx = (cond_coords[:, 1:4].astype(np.float32) + f32(0.5)) * f32(0.01)
    ck = _morton3(cond_coords[:, 1:4])
    corder = np.argsort(ck, kind='stable')
    bmin, bmax = cx.min(0), cx.max(0)
    clamped = np.clip(nx, bmin, bmax)
    D = np.linalg.norm(nx - clamped, axis=1)
    dens = M / float(np.prod((bmax - bmin).astype(np.float64)))
    outdims = ((nx < bmin) | (nx > bmax)).sum(1)
    ceff = CSAFE * np.maximum(1.0, 2.0 ** (outdims - 1))
    vol = ceff / dens
    delta_face = np.sqrt(vol / (np.pi * np.maximum(D, 1e-9)))
    delta_sph = (vol * 3.0 / (4.0 * np.pi)) ** (1.0 / 3.0)
    delta = np.where(D > 0.3, delta_face, delta_sph)
    rr = np.sqrt(2 * D * delta + delta ** 2)

    ngrid = np.clip(np.round(clamped / 0.01 - 0.5), 0, 1023).astype(np.int64)
    nk = _morton3(ngrid)
    norder = np.argsort(nk, kind='stable')

    bidx = corder.reshape(NB, BS)
    bco = cx[bidx]
    blo, bhi = bco.min(1), bco.max(1)

    ntile = N // 128
    sel_blocks, w_tiles = [], []
    for t in range(ntile):
        mem = norder[t * 128:(t + 1) * 128]
        s, r = clamped[mem], rr[mem]
        d = (np.maximum(blo[None] - s[:, None], 0)
             + np.maximum(s[:, None] - bhi[None], 0))
        need = (np.linalg.norm(d, axis=2) <= r[:, None]).any(0)
        sel = np.where(need)[0]
        sel_blocks.append(sel)
        w_tiles.append(len(sel) * BS)
    w_tiles = np.array(w_tiles)
    pi = np.argsort(-w_tiles, kind='stable')      # tiles sorted by W desc
    budgets = tuple(int(-(-w_tiles[pi[8 * k]] // GR)) * GR for k in range(NTILES))
    assign = pi.reshape(NTILES, NCORES)
    return dict(corder=corder, norder=norder, sel_blocks=sel_blocks,
                budgets=budgets, assign=assign)


_CACHE = {}


def _build_program(budgets):
    key = ('nc', budgets)
    if key in _CACHE:
        return _CACHE[key]
    nc = bacc.Bacc("TRN2", target_bir_lowering=False, debug=False,
                   num_devices=NCORES)
    dt = mybir.dt
    WMAX = max(budgets)
    WTOT = sum(budgets)
    pools = [b // 128 for b in budgets]            # pool width per slot (8/granule)
    PTOT = sum(pools)
    PMAX = max(pools)
    woff = np.cumsum([0] + [b for b in budgets])
    poff = np.cumsum([0] + pools)

    def din(name, shape, dtype):
        return nc.dram_tensor(name, shape, dtype, kind="ExternalInput").ap()

    lhsT = din('lhsT', [19, NSHARD], dt.bfloat16)
    rhsw = din('rhsw', [19, WTOT], dt.bfloat16)
    nodex = din('nodex', [128, NTILES * 3], dt.float32)
    packed = din('packed', [M, PACK], dt.float32)
    inv0t = din('inv0t', [128, PTOT], dt.float32)
    d01t = din('d01t', [128, PTOT], dt.float32)
    eye = din('eye', [128, 128], dt.float32)
    wp = din('wp', [128, 512], dt.float32)
    wl1 = din('wl1', [128, 512], dt.float32)
    wl2 = din('wl2', [128, 512], dt.float32)
    bproj = din('bproj', [128, 2], dt.float32)
    bl1 = din('bl1', [128, 2], dt.float32)
    bcomb = din('bcomb', [128, 2], dt.float32)
    wt1 = din('wt1', [EMBED, EMBED], dt.float32)
    wt2 = din('wt2', [EMBED, C], dt.float32)
    bt1 = din('bt1', [EMBED, 1], dt.float32)
    freqs = din('freqs', [EMBED, 1], dt.float32)
    shifts = din('shifts', [EMBED, 1], dt.float32)
    tval = din('tval', [EMBED, 1], dt.float32)
    out = nc.dram_tensor('out', [NSHARD, C], dt.float32, kind="ExternalOutput").ap()

    with TileContext(nc) as tc, \
            tc.tile_pool(name="const", bufs=1) as cpool, \
            tc.tile_pool(name="work", bufs=2) as wpool, \
            tc.tile_pool(name="psum", bufs=2, space="PSUM") as ppool:

        # ---- constants to SBUF
        lhs_sb = cpool.tile([19, NSHARD], dt.bfloat16, tag="lhs")
        nc.sync.dma_start(out=lhs_sb[:], in_=lhsT)
        nodex_sb = cpool.tile([128, NTILES * 3], dt.float32, tag="nodex")
        nc.sync.dma_start(out=nodex_sb[:], in_=nodex)
        inv0_sb = cpool.tile([128, PTOT], dt.float32, tag="inv0")
        nc.sync.dma_start(out=inv0_sb[:], in_=inv0t)
        d01_sb = cpool.tile([128, PTOT], dt.float32, tag="d01")
        nc.sync.dma_start(out=d01_sb[:], in_=d01t)
        eye_sb = cpool.tile([128, 128], dt.float32, tag="eye")
        nc.sync.dma_start(out=eye_sb[:], in_=eye)
        wp_sb = cpool.tile([128, 512], dt.float32, tag="wp")
        nc.sync.dma_start(out=wp_sb[:], in_=wp)
        wl1_sb = cpool.tile([128, 512], dt.float32, tag="wl1")
        nc.sync.dma_start(out=wl1_sb[:], in_=wl1)
        wl2_sb = cpool.tile([128, 512], dt.float32, tag="wl2")
        nc.sync.dma_start(out=wl2_sb[:], in_=wl2)
        bproj_sb = cpool.tile([128, 2], dt.float32, tag="bproj")
        nc.sync.dma_start(out=bproj_sb[:], in_=bproj)
        bl1_sb = cpool.tile([128, 2], dt.float32, tag="bl1")
        nc.sync.dma_start(out=bl1_sb[:], in_=bl1)
        bcomb_sb = cpool.tile([128, 2], dt.float32, tag="bcomb")
        nc.sync.dma_start(out=bcomb_sb[:], in_=bcomb)
        wt1_sb = cpool.tile([EMBED, EMBED], dt.float32, tag="wt1")
        nc.sync.dma_start(out=wt1_sb[:], in_=wt1)
        wt2_sb = cpool.tile([EMBED, C], dt.float32, tag="wt2")
        nc.sync.dma_start(out=wt2_sb[:], in_=wt2)
        bt1_sb = cpool.tile([EMBED, 1], dt.float32, tag="bt1")
        nc.sync.dma_start(out=bt1_sb[:], in_=bt1)
        fr_sb = cpool.tile([EMBED, 1], dt.float32, tag="fr")
        nc.sync.dma_start(out=fr_sb[:], in_=freqs)
        sh_sb = cpool.tile([EMBED, 1], dt.float32, tag="sh")
        nc.sync.dma_start(out=sh_sb[:], in_=shifts)
        t_sb = cpool.tile([EMBED, 1], dt.float32, tag="t1x1")
        nc.sync.dma_start(out=t_sb[:], in_=tval)

        # ---- t branch -> fincol [128, 2]
        e = cpool.tile([EMBED, 1], dt.float32, tag="e")
        nc.vector.tensor_mul(e[:], t_sb[:], fr_sb[:])
        nc.vector.tensor_add(e[:], e[:], sh_sb[:])
        ki = cpool.tile([EMBED, 1], dt.int32, tag="ki")
        kf = cpool.tile([EMBED, 1], dt.float32, tag="kf")
        nc.vector.tensor_scalar(kf[:], e[:], float(1.0 / (2 * PI)), None, op0=OP.mult)
        nc.vector.tensor_copy(out=ki[:], in_=kf[:])
        nc.vector.tensor_copy(out=kf[:], in_=ki[:])
        nc.vector.tensor_scalar(kf[:], kf[:], float(2 * PI), None, op0=OP.mult)
        nc.vector.tensor_sub(e[:], e[:], kf[:])
        gt = cpool.tile([EMBED, 1], dt.float32, tag="gt")
        nc.vector.tensor_scalar(gt[:], e[:], float(PI), None, op0=OP.is_gt)
        nc.vector.tensor_scalar(gt[:], gt[:], float(2 * PI), None, op0=OP.mult)
        nc.vector.tensor_sub(e[:], e[:], gt[:])
        emb_sb = cpool.tile([EMBED, 1], dt.float32, tag="emb")
        nc.scalar.activation(emb_sb[:], e[:], AF.Sin)
        ps_t1 = ppool.tile([EMBED, 1], dt.float32, tag="mm", bufs=1)
        nc.tensor.matmul(ps_t1[:], lhsT=wt1_sb[:], rhs=emb_sb[:], start=True, stop=True)
        h96 = cpool.tile([EMBED, 1], dt.float32, tag="h96")
        nc.scalar.activation(h96[:], ps_t1[:], AF.Identity, bias=bt1_sb[:, 0:1])
        h96b = cpool.tile([EMBED, 1], dt.float32, tag="h96b")
        nc.vector.tensor_scalar(h96b[:], h96[:], 0.1, None, op0=OP.mult)
        nc.vector.tensor_max(h96b[:], h96b[:], h96[:])
        fincol = cpool.tile([128, 2], dt.float32, tag="fincol")
        for d in range(2):
            ps_t2 = ppool.tile([128, 1], dt.float32, tag="mm", bufs=1)
            nc.tensor.matmul(ps_t2[:], lhsT=wt2_sb[:, d * 128:(d + 1) * 128],
                             rhs=h96b[:], start=True, stop=True)
            nc.scalar.activation(fincol[:, d:d + 1], ps_t2[:], AF.Identity,
                                 bias=bcomb_sb[:, d:d + 1])

        c32k = cpool.tile([128, NCAND], dt.float32, tag="c32k")
        nc.vector.memset(c32k[:], 32768.0)

        def stageA(t):
            st = {}
            Wk = budgets[t]
            Gk = Wk // GR
            Pk = pools[t]
            lt = lhs_sb[:, t * 128:(t + 1) * 128]
            rhs_sb = wpool.tile([19, WMAX], dt.bfloat16, tag="rhsw", name="rhsw",
                                bufs=2)
            for j in range(0, Wk, 4096):
                w = min(4096, Wk - j)
                nc.sync.dma_start(out=rhs_sb[:, j:j + w],
                                  in_=rhsw[:, woff[t] + j:woff[t] + j + w])
            pool_vals = wpool.tile([128, PMAX], dt.float32, tag="pvals",
                                   name="pvals", bufs=2)
            pool_lidx = wpool.tile([128, PMAX], dt.uint16, tag="plidx",
                                   name="plidx", bufs=2)
            for g in range(Gk):
                ps_d = ppool.tile([128, GR], dt.float32, tag="dist", name="psd",
                                  bufs=2)
                for q in range(2):
                    nc.tensor.matmul(ps_d[:, q * 512:(q + 1) * 512], lhsT=lt,
                                     rhs=rhs_sb[:, g * GR + q * 512:g * GR + (q + 1) * 512],
                                     start=True, stop=True)
                rowb = wpool.tile([128, GR], dt.float32, tag="rowb", name="rowb",
                                  bufs=4)
                nc.scalar.activation(rowb[:], ps_d[:], AF.Identity)
                nc.vector.max(out=pool_vals[:, g * 8:(g + 1) * 8], in_=rowb[:])
                nc.vector.max_index(out=pool_lidx[:, g * 8:(g + 1) * 8],
                                    in_max=pool_vals[:, g * 8:(g + 1) * 8],
                                    in_values=rowb[:])
            pv = pool_vals[:, 0:Pk]
            lidxf = wpool.tile([128, PMAX], dt.float32, tag="lidxf", name="lidxf")
            nc.vector.tensor_copy(out=lidxf[:, 0:Pk], in_=pool_lidx[:, 0:Pk])
            # piecewise-affine index: pinv = inv0 - lidx - (lidx>=512)*d01
            tge = wpool.tile([128, PMAX], dt.float32, tag="tge", name="tge")
            nc.vector.tensor_scalar(tge[:, 0:Pk], lidxf[:, 0:Pk], 512.0, None,
                                    op0=OP.is_ge)
            pinv = wpool.tile([128, PMAX], dt.float32, tag="pinv", name="pinv")
            nc.vector.tensor_sub(pinv[:, 0:Pk],
                                 inv0_sb[:, poff[t]:poff[t] + Pk],
                                 lidxf[:, 0:Pk])
            nc.vector.tensor_mul(tge[:, 0:Pk], tge[:, 0:Pk],
                                 d01_sb[:, poff[t]:poff[t] + Pk])
            nc.vector.tensor_sub(pinv[:, 0:Pk], pinv[:, 0:Pk], tge[:, 0:Pk])
            # top-10 by value: v8 (ranks 1-8), v8b[:,0:2] (ranks 9-10)
            v8 = wpool.tile([128, 8], dt.float32, tag="v8", name="v8")
            pv2 = wpool.tile([128, PMAX], dt.float32, tag="pv2", name="pv2")
            pv3 = wpool.tile([128, PMAX], dt.float32, tag="pv3", name="pv3")
            nc.vector.max(out=v8[:], in_=pv)
            nc.vector.match_replace(out=pv2[:, 0:Pk], in_to_replace=v8[:],
                                    in_values=pv, imm_value=NEG)
            v8b = wpool.tile([128, 8], dt.float32, tag="v8b", name="v8b")
            nc.vector.max(out=v8b[:], in_=pv2[:, 0:Pk])
            v10p = wpool.tile([128, 8], dt.float32, tag="v10p", name="v10p")
            nc.vector.memset(v10p[:], NEG)
            nc.vector.tensor_copy(out=v10p[:, 0:2], in_=v8b[:, 0:2])
            nc.vector.match_replace(out=pv3[:, 0:Pk], in_to_replace=v10p[:],
                                    in_values=pv2[:, 0:Pk], imm_value=NEG)
            maskp = wpool.tile([128, PMAX], dt.float32, tag="maskp", name="maskp")
            nc.vector.tensor_tensor(out=maskp[:, 0:Pk], in0=pv3[:, 0:Pk],
                                    in1=pv, op=OP.not_equal)
            nc.vector.tensor_mul(maskp[:, 0:Pk], maskp[:, 0:Pk], pinv[:, 0:Pk])
            i8a = wpool.tile([128, 8], dt.float32, tag="i8a", name="i8a")
            nc.vector.max(out=i8a[:], in_=maskp[:, 0:Pk])
            mv2 = wpool.tile([128, PMAX], dt.float32, tag="mv2", name="mv2")
            nc.vector.match_replace(out=mv2[:, 0:Pk], in_to_replace=i8a[:],
                                    in_values=maskp[:, 0:Pk], imm_value=0.0)
            i8b = wpool.tile([128, 8], dt.float32, tag="i8b", name="i8b")
            nc.vector.max(out=i8b[:], in_=mv2[:, 0:Pk])
            inv10 = wpool.tile([128, NCAND], dt.float32, tag="inv10", name="inv10")
            nc.vector.tensor_copy(out=inv10[:, 0:8], in_=i8a[:])
            nc.vector.tensor_copy(out=inv10[:, 8:10], in_=i8b[:, 0:2])
            g10f = wpool.tile([128, NCAND], dt.float32, tag="g10f", name="g10f")
            nc.vector.tensor_sub(g10f[:], c32k[:], inv10[:])
            g10u = wpool.tile([128, NCAND], dt.uint32, tag="g10u", name="g10u")
            nc.vector.tensor_copy(out=g10u[:], in_=g10f[:])
            gp = wpool.tile([128, NCAND, PACK], dt.float32, tag="gp", name="gp",
                            bufs=3)
            for k in range(NCAND):
                nc.gpsimd.indirect_dma_start(
                    out=gp[:, k, :], out_offset=None, in_=packed,
                    in_offset=bass.IndirectOffsetOnAxis(ap=g10u[:, k:k + 1], axis=0))
            st['gp'] = gp
            return st

        def stageBpre(t, st, fTB, slot):
            gp = st['gp']
            xs = [nodex_sb[:, t * 3 + k: t * 3 + k + 1] for k in range(3)]
            dcol = [wpool.tile([128, NCAND], dt.float32, tag=f"d{k}",
                               name=f"dcol{k}") for k in range(3)]
            for k in range(3):
                nc.vector.tensor_scalar(dcol[k][:], gp[:, :, k + 1], xs[k],
                                        None, op0=OP.subtract)
            acc = wpool.tile([128, NCAND], dt.float32, tag="acc", name="acc")
            nc.gpsimd.tensor_mul(acc[:], dcol[0][:], dcol[0][:])
            tt = [wpool.tile([128, NCAND], dt.float32, tag=f"t{i}",
                             name=f"tt{i}") for i in range(6)]
            for k in (1, 2):
                d = dcol[k]
                T0, T1, T2, T3, T4, T5 = tt
                nc.vector.tensor_scalar(T0[:], d[:], 4097.0, None, op0=OP.mult)
                nc.gpsimd.tensor_sub(T1[:], T0[:], d[:])
                nc.gpsimd.tensor_sub(T0[:], T0[:], T1[:])
                nc.gpsimd.tensor_sub(T1[:], d[:], T0[:])
                nc.gpsimd.tensor_mul(T2[:], d[:], d[:])
                nc.gpsimd.tensor_mul(T3[:], T0[:], T0[:])
                nc.gpsimd.tensor_sub(T3[:], T3[:], T2[:])
                nc.gpsimd.tensor_add(T4[:], T1[:], T1[:])
                nc.gpsimd.tensor_mul(T4[:], T0[:], T4[:])
                nc.gpsimd.tensor_add(T3[:], T3[:], T4[:])
                nc.gpsimd.tensor_mul(T4[:], T1[:], T1[:])
                nc.gpsimd.tensor_add(T3[:], T3[:], T4[:])
                nc.gpsimd.tensor_add(T4[:], T2[:], acc[:])
                nc.gpsimd.tensor_sub(T5[:], T4[:], T2[:])
                nc.gpsimd.tensor_sub(T0[:], T4[:], T5[:])
                nc.gpsimd.tensor_sub(T0[:], T2[:], T0[:])
                nc.gpsimd.tensor_sub(T1[:], acc[:], T5[:])
                nc.gpsimd.tensor_add(T0[:], T0[:], T1[:])
                nc.gpsimd.tensor_add(T0[:], T0[:], T3[:])
                nc.gpsimd.tensor_add(acc[:], T4[:], T0[:])
            nd2 = wpool.tile([128, NCAND], dt.float32, tag="nd2", name="nd2")
            nc.vector.tensor_scalar(nd2[:], acc[:], -1.0, None, op0=OP.mult)
            v8f = wpool.tile([128, 8], dt.float32, tag="v8f", name="v8f")
            nc.vector.max(out=v8f[:], in_=nd2[:])
            refb = wpool.tile([128, NCAND], dt.float32, tag="refb", name="refb")
            nc.vector.match_replace(out=refb[:], in_to_replace=v8f[:],
                                    in_values=nd2[:], imm_value=3e38)
            mask2 = wpool.tile([128, NCAND], dt.float32, tag="mask2", name="mask2")
            nc.vector.tensor_tensor(out=mask2[:], in0=refb[:], in1=nd2[:],
                                    op=OP.not_equal)
            dist = wpool.tile([128, NCAND], dt.float32, tag="dist16", name="dist16")
            nc.scalar.activation(dist[:], acc[:], AF.Sqrt)
            nc.vector.tensor_scalar_max(dist[:], dist[:], 1e-6)
            wr = wpool.tile([128, NCAND], dt.float32, tag="wr", name="wr")
            nc.vector.reciprocal(out=wr[:], in_=dist[:])
            nc.vector.tensor_mul(wr[:], wr[:], mask2[:])
            wsum = wpool.tile([128, 1], dt.float32, tag="wsum", name="wsum")
            nc.vector.tensor_reduce(out=wsum[:], in_=wr[:],
                                    axis=mybir.AxisListType.X, op=OP.add)
            wsr = wpool.tile([128, 1], dt.float32, tag="wsr", name="wsr")
            nc.vector.reciprocal(out=wsr[:], in_=wsum[:])
            wn = wpool.tile([128, NCAND], dt.float32, tag="wn", name="wn")
            nc.vector.tensor_scalar(wn[:], wr[:], wsr[:, 0:1], None, op0=OP.mult)

            facc = wpool.tile([128, C], dt.float32, tag="facc", name="facc")
            nc.vector.tensor_scalar(facc[:], gp[:, 0, 4:4 + C], wn[:, 0:1],
                                    None, op0=OP.mult)
            for k in range(1, NCAND):
                nc.vector.scalar_tensor_tensor(
                    out=facc[:], in0=gp[:, k, 4:4 + C], scalar=wn[:, k:k + 1],
                    in1=facc[:], op0=OP.mult, op1=OP.add)
            for half_i in range(2):
                ps_tr = ppool.tile([128, 128], dt.float32, tag="tr", name="pstr", bufs=1)
                nc.tensor.matmul(ps_tr[:],
                                 lhsT=facc[:, half_i * 128:(half_i + 1) * 128],
                                 rhs=eye_sb[:], is_transpose=True,
                                 start=True, stop=True)
                nc.scalar.activation(fTB[:, half_i, slot, :], ps_tr[:],
                                     AF.Identity)

        def stageBmlp(g, fTB):
            def denseB(src, wpack, bias_sb, leaky, tag):
                o = wpool.tile([128, 2, 4, 128], dt.float32, tag=tag, name=tag)
                for dh in range(2):
                    ps = ppool.tile([128, 512], dt.float32, tag="mmB", name="mmB")
                    for ct in range(2):
                        nc.tensor.matmul(
                            ps[:], lhsT=wpack[:, (ct * 2 + dh) * 128:(ct * 2 + dh + 1) * 128],
                            rhs=src[:, ct, :, :],
                            start=(ct == 0), stop=(ct == 1))
                    nc.scalar.activation(o[:, dh, :, :], ps[:],
                                         AF.Identity, bias=bias_sb[:, dh:dh + 1])
                if leaky:
                    tmp = wpool.tile([128, 2, 4, 128], dt.float32, tag=tag + "lk",
                                     name=tag + "lk")
                    nc.vector.tensor_scalar(tmp[:], o[:], 0.1, None, op0=OP.mult)
                    nc.vector.tensor_max(o[:], o[:], tmp[:])
                return o

            mTB = denseB(fTB, wp_sb, bproj_sb, False, "mTB")
            h1B = denseB(mTB, wl1_sb, bl1_sb, True, "h1B")
            pTB = denseB(h1B, wl2_sb, fincol, False, "pTB")
            for ti in range(4):
                t = g * 4 + ti
                osb = wpool.tile([128, C], dt.float32, tag="osb", name="osb",
                                 bufs=3)
                for dth in range(2):
                    ps_tr = ppool.tile([128, 128], dt.float32, tag="tr",
                                       name="pstr2", bufs=1)
                    nc.tensor.matmul(ps_tr[:], lhsT=pTB[:, dth, ti, :],
                                     rhs=eye_sb[:], is_transpose=True,
                                     start=True, stop=True)
                    nc.scalar.activation(osb[:, dth * 128:(dth + 1) * 128],
                                         ps_tr[:], AF.Identity)
                nc.sync.dma_start(out=out[t * 128:(t + 1) * 128, :], in_=osb[:])

        pend = []
        fTBs = {}
        for t in range(NTILES + 2):
            if t < NTILES:
                pend.append(stageA(t))
            if t >= 2:
                tb = t - 2
                gidx = tb // 4
                if tb % 4 == 0:
                    fTBs[gidx] = wpool.tile([128, 2, 4, 128], dt.float32,
                                            tag="fTB", name="fTB", bufs=2)
                stageBpre(tb, pend.pop(0), fTBs[gidx], tb % 4)
                if tb % 4 == 3:
                    stageBmlp(gidx, fTBs.pop(gidx))

    nc.compile()
    _CACHE[key] = nc
    return nc


# ---------------------------------------------------------------- host entry
def kernel(node_coords, cond_coords, cond_feats, t,
           W_proj, b_proj, W_l1, b_l1, W_l2, b_l2, W_t1, b_t1, W_t2, b_t2):
    node_coords = np.asarray(node_coords)
    cond_coords = np.asarray(cond_coords)
    cond_feats = np.asarray(cond_feats, dtype=np.float32)
    mc = np.float32(node_coords.astype(np.float32).max())

    plan = _host_plan(node_coords, cond_coords)
    corder = plan['corder']
    norder = plan['norder']
    sel_blocks = plan['sel_blocks']
    budgets = plan['budgets']
    assign = plan['assign']          # [NTILES, NCORES]: global tile id
    pools = [b // 128 for b in budgets]

    part_c = _transform(cond_coords, 1.0, 0.01, mc)
    packed = np.ascontiguousarray(
        np.concatenate([part_c, cond_feats], 1)[corder])
    LHS, RHS = _split_rows(node_coords, cond_coords)
    RHSs = np.ascontiguousarray(RHS[:, corder])      # sorted cond order
    full_c = _transform(node_coords, 16.0, 0.05, mc)
    fill = _filler_col()

    eye = np.eye(128, dtype=f32)
    freqs1 = np.exp(np.arange(HALF, dtype=np.float32) *
                    f32(-math.log(10000.0) / (HALF - 1))).astype(f32)
    freqs = np.concatenate([freqs1, freqs1])
    shifts = np.concatenate([np.zeros(HALF, f32), np.full(HALF, PI / 2, f32)])

    nc = _build_program(budgets)
    in_maps = []
    node_order_core = []
    for ci in range(NCORES):
        tiles = [assign[k, ci] for k in range(NTILES)]
        nodes = np.concatenate([norder[g * 128:(g + 1) * 128] for g in tiles])
        node_order_core.append(nodes)
        rhs_parts, inv0_parts, d01_parts = [], [], []
        for k, g in enumerate(tiles):
            sel = sel_blocks[g]
            cols = np.concatenate([RHSs[:, b * BS:(b + 1) * BS] for b in sel], 1)
            nfill = budgets[k] - cols.shape[1]
            if nfill > 0:
                cols = np.concatenate(
                    [cols, np.tile(fill[:, None], (1, nfill))], 1)
            rhs_parts.append(cols)
            i0 = np.empty(pools[k], f32)
            d01 = np.empty(pools[k], f32)
            for gi in range(budgets[k] // GR):
                b0 = sel[2 * gi] * BS if 2 * gi < len(sel) else 0
                b1 = sel[2 * gi + 1] * BS if 2 * gi + 1 < len(sel) else 0
                i0[gi * 8:(gi + 1) * 8] = f32(32768.0 - b0)
                d01[gi * 8:(gi + 1) * 8] = f32(b1 - b0 - 512)
            inv0_parts.append(i0)
            d01_parts.append(d01)
        rhsw = np.ascontiguousarray(np.concatenate(rhs_parts, 1))
        inv0 = np.tile(np.concatenate(inv0_parts)[None, :], (128, 1))
        d01 = np.tile(np.concatenate(d01_parts)[None, :], (128, 1))
        nodex = np.ascontiguousarray(
            full_c[nodes, 1:4].reshape(NTILES, 128, 3).transpose(1, 0, 2)
            .reshape(128, NTILES * 3))
        in_maps.append({
            'lhsT': np.ascontiguousarray(LHS[:, nodes]),
            'rhsw': rhsw,
            'nodex': nodex,
            'packed': packed,
            'inv0t': np.ascontiguousarray(inv0),
            'd01t': np.ascontiguousarray(d01),
            'eye': eye,
            'wp': _pack_w(np.asarray(W_proj, dtype=f32)),
            'wl1': _pack_w(np.asarray(W_l1, dtype=f32)),
            'wl2': _pack_w(np.asarray(W_l2, dtype=f32)),
            'bproj': np.asarray(b_proj, f32).reshape(2, 128).T.copy(),
            'bl1': np.asarray(b_l1, f32).reshape(2, 128).T.copy(),
            'bcomb': (np.asarray(b_l2, f32) + np.asarray(b_t2, f32)).reshape(2, 128).T.copy(),
            'wt1': np.ascontiguousarray(np.asarray(W_t1, f32).T),
            'wt2': np.ascontiguousarray(np.asarray(W_t2, f32).T),
            'bt1': np.asarray(b_t1, f32).reshape(EMBED, 1).copy(),
            'freqs': freqs.reshape(EMBED, 1).copy(),
            'shifts': shifts.reshape(EMBED, 1).copy(),
            'tval': np.full((EMBED, 1), np.asarray(t, f32).reshape(()), f32),
        })
    res = bass_utils.run_bass_kernel_spmd(nc, in_maps, core_ids=list(range(NCORES)))
    _CACHE['last_result'] = res
    out_full = np.empty((N, C), np.float32)
    for ci in range(NCORES):
        out_full[node_order_core[ci]] = res.results[ci]['out']
    return out_full
